# revision 1
# baseline (speedup 1.0000x reference)
"""Causal self-attention (GQA, rope, qk-rmsnorm) Trainium2 kernel, 8 NeuronCores.

Sharding: core = (b, g), b = core // 4 (batch), g = core % 4.
Each core handles query row-chunks {g, 4+g, 8+g, 12+g} (128 rows each) of its
batch: computes Q for those 512 rows, K/V for all keys it needs (duplicated),
attention for all 16 heads, and its 512-row slice of the output projection.
Host gathers row slices. The program is identical on all cores (SPMD); all
per-core variation comes through the input shards.

Slot c (c = 0..3) covers query chunk 4c+g with keys [0, 512*(c+1)) — uniform
across cores; causal masking inside the last 512 keys comes from a
host-provided additive mask shard.
"""

import sys

if "/opt/trn_rl_repo" not in sys.path:
    sys.path.insert(0, "/opt/trn_rl_repo")

import numpy as np

B, T, C = 2, 2048, 2048
NH, NKV = 16, 4
HD = C // NH  # 128
P = 128
NT = T // P            # 16 token tiles per batch
NCT = C // P           # 16 contraction tiles
QROWS = 512            # own query rows per core
NQT = QROWS // P       # 4 own token tiles
KLEN = [512, 1024, 1536, 2048]   # keys per slot
SCALE = 1.0 / float(np.sqrt(HD))
EPS = float(np.finfo(np.float32).eps)
NEG = -1.0e9

_CACHE = {}


def _chunks(g):
    return [g, 4 + g, 8 + g, 12 + g]


def _rows(g):
    return np.concatenate([np.arange(ch * P, (ch + 1) * P) for ch in _chunks(g)])


def _qmask_t(g):
    """Additive mask, transposed layout: (slot c, sub s, k_in_sub i, q j).

    For slot c the score tile is S^T[k, q] with k in [0, KLEN[c]) and q the
    128 rows of chunk 4c+g. Only keys in the last 512 of the slot can be
    invalid; mask[c, s, i, j] = 0 if key (KLEN[c]-512 + s*128 + i) <= query
    (128*(4c+g) + j) else NEG.
    """
    m = np.zeros((4, 4, P, P), np.float32)
    for c in range(4):
        k0 = KLEN[c] - 512
        r0 = (4 * c + g) * P
        k = k0 + np.arange(512)[:, None]          # (512, 1)
        q = r0 + np.arange(P)[None, :]            # (1, 128)
        m[c] = np.where(k <= q, 0.0, NEG).reshape(4, P, P)
    return m


def _build():
    import concourse.bacc as bacc
    import concourse.bass as bass
    import concourse.mybir as mybir
    import concourse.tile as tile
    from concourse.masks import make_identity

    f32 = mybir.dt.float32
    bf16 = mybir.dt.bfloat16
    AF = mybir.ActivationFunctionType
    OP = mybir.AluOpType
    AX = mybir.AxisListType

    nc = bacc.Bacc("TRN2", target_bir_lowering=False, debug=False, num_devices=8)

    xf = nc.dram_tensor("xf", [T, C], f32, kind="ExternalInput").ap()
    xo = nc.dram_tensor("xo", [QROWS, C], f32, kind="ExternalInput").ap()
    cosf = nc.dram_tensor("cosf", [T, HD // 2], f32, kind="ExternalInput").ap()
    sinf = nc.dram_tensor("sinf", [T, HD // 2], f32, kind="ExternalInput").ap()
    coso = nc.dram_tensor("coso", [QROWS, HD // 2], f32, kind="ExternalInput").ap()
    sino = nc.dram_tensor("sino", [QROWS, HD // 2], f32, kind="ExternalInput").ap()
    wq = nc.dram_tensor("wq", [C, C], f32, kind="ExternalInput").ap()
    wk = nc.dram_tensor("wk", [C, NKV * HD], f32, kind="ExternalInput").ap()
    wv = nc.dram_tensor("wv", [C, NKV * HD], f32, kind="ExternalInput").ap()
    wo = nc.dram_tensor("wo", [C, C], f32, kind="ExternalInput").ap()
    qm = nc.dram_tensor("qm", [4, 4, P, P], f32, kind="ExternalInput").ap()
    yo = nc.dram_tensor("yo", [QROWS, C], f32, kind="ExternalOutput").ap()

    def bcast4(ap2d):
        # [128, 64] -> [128, 4, 64] with middle step 0 (replicate across heads)
        return bass.AP(
            tensor=ap2d.tensor,
            offset=ap2d.offset,
            ap=[ap2d.ap[0], [0, 4], ap2d.ap[1]],
        )

    with tile.TileContext(nc) as tc:
        with (
            tc.tile_pool(name="singles", bufs=1) as singles,
            tc.tile_pool(name="big", bufs=1) as bigpool,
            tc.tile_pool(name="xb", bufs=2) as xbpool,
            tc.tile_pool(name="wsl", bufs=2) as wslpool,
            tc.tile_pool(name="cs", bufs=2) as cspool,
            tc.tile_pool(name="epi", bufs=2) as epipool,
            tc.tile_pool(name="qh", bufs=3) as qhpool,
            tc.tile_pool(name="wo3", bufs=2) as wopool,
            tc.tile_pool(name="pt", bufs=4) as ptpool,
            tc.tile_pool(name="smallf", bufs=2) as smallf,
            tc.tile_pool(name="outs", bufs=2) as outpool,
            tc.tile_pool(name="psS", bufs=3, space="PSUM") as psS,
            tc.tile_pool(name="psY", bufs=3, space="PSUM") as psY,
            tc.tile_pool(name="psD", bufs=2, space="PSUM") as psD,
        ):
            ident = singles.tile([P, P], bf16)
            make_identity(nc, ident)
            identf = singles.tile([P, P], f32)
            make_identity(nc, identf)
            ones128 = singles.tile([P, P], bf16)
            nc.vector.memset(ones128, 1.0)
            eps_q = singles.tile([P, 1], f32)
            nc.vector.memset(eps_q, EPS)
            eps_k = singles.tile([P, 1], f32)
            nc.vector.memset(eps_k, HD * EPS)

            # persistent big SBUF tensors
            qT = bigpool.tile([P, NH, QROWS], bf16, tag="qT")      # [d, h, q]
            kT = bigpool.tile([P, NKV, T], bf16, tag="kT")         # [d, kvh, k]
            vA = bigpool.tile([P, NT, NKV, HD], bf16, tag="vA")    # [ktok, tt, kvh, d]
            yT = bigpool.tile([P, NCT, QROWS], bf16, tag="yT")     # [d, ct, q]
            qmask = singles.tile([P, 4, 4, P], bf16)               # [ki, c, sub, q]
            nc.gpsimd.dma_start(out=qmask, in_=qm.rearrange("c s i j -> i c s j"))

            # ---------------- helpers ----------------
            def x_transpose(src_ap, dst4, dst_col, name):
                """Load one 128-row strip of x (f32 dram) via HWDGE, transpose
                (f32) into the 4 [P, 4, width] bf16 dst tiles at dst_col."""
                for h2 in range(2):
                    xb = xbpool.tile([P, 8 * P], f32, tag="xb", name=f"xb{name}{h2}")
                    nc.sync.dma_start(
                        out=xb, in_=src_ap[:, h2 * 1024:(h2 + 1) * 1024]
                    )
                    for gr in (2 * h2, 2 * h2 + 1):
                        ptr = psY.tile([P, 512], f32, tag="Y", name=f"ptr{name}{gr}")
                        for j in range(4):
                            ctl = 4 * gr + j - 8 * h2
                            nc.tensor.transpose(
                                ptr[:, j * P:(j + 1) * P],
                                xb[:, ctl * P:(ctl + 1) * P],
                                identf,
                            )
                        nc.scalar.copy(
                            dst4[gr][:, :, dst_col:dst_col + P],
                            ptr.rearrange("p (s n) -> p s n", s=4),
                        )

            def load_w_slab(w_ap, col0, name):
                """One 512-col slab of a weight, as [128, 16, 512] bf16."""
                wsl = wslpool.tile([P, NCT, 512], bf16, tag="wsl", name=name)
                for gr in range(4):
                    nc.gpsimd.dma_start(
                        out=wsl[:, 4 * gr:4 * gr + 4, :],
                        in_=w_ap[:, col0:col0 + 512].rearrange(
                            "(a p) n -> p a n", p=P
                        )[:, 4 * gr:4 * gr + 4, :],
                    )
                return wsl

            pending = []  # delayed PE transpose packs (2-deep pipeline)

            def drain_pending(keep=0):
                while len(pending) > keep:
                    pending.pop(0)()

            def rope_rms(ps, cos4, sin4, out_bf, eps_ap, sqrt_scale):
                """ps: [128, 512] psum f32 (4 heads). Writes normalized bf16
                rope output to out_bf [128, 4, 128]."""
                v3 = ps.rearrange("p (h d) -> p h d", h=4)
                ro = epipool.tile([P, 4, HD], f32, tag="ro", name="ro")
                cs = epipool.tile([P, 4, HD], f32, tag="cs", name="cs")
                sn = epipool.tile([P, 4, HD], f32, tag="sn", name="sn")
                nc.vector.tensor_tensor(cs[:, :, 0:64], v3[:, :, 0:64], cos4, op=OP.mult)
                nc.vector.tensor_tensor(cs[:, :, 64:128], v3[:, :, 64:128], cos4, op=OP.mult)
                nc.vector.tensor_tensor(sn[:, :, 0:64], v3[:, :, 0:64], sin4, op=OP.mult)
                nc.vector.tensor_tensor(sn[:, :, 64:128], v3[:, :, 64:128], sin4, op=OP.mult)
                nc.vector.tensor_tensor(ro[:, :, 0:64], cs[:, :, 0:64], sn[:, :, 64:128], op=OP.add)
                nc.vector.tensor_sub(ro[:, :, 64:128], cs[:, :, 64:128], sn[:, :, 0:64])
                ss = smallf.tile([P, 4], f32, tag="ss", name="ss")
                sq = epipool.tile([P, 4, HD], f32, tag="cs", name="sq")
                nc.vector.tensor_tensor(sq, ro, ro, op=OP.mult)
                nc.vector.reduce_sum(ss, sq, axis=AX.X)
                rms = smallf.tile([P, 4], f32, tag="rms", name="rms")
                nc.scalar.activation(rms, ss, AF.Sqrt, bias=eps_ap, scale=sqrt_scale)
                rinv = smallf.tile([P, 4], f32, tag="rms", name="rinv")
                nc.vector.reciprocal_approx_fast(rinv, rms)
                for hh in range(4):
                    nc.vector.tensor_scalar_mul(
                        out_bf[:, hh, :], ro[:, hh, :], rinv[:, hh:hh + 1]
                    )

            def pack_transpose(src_bf, dst):
                """src_bf [128, 4, 128] bf16 -> 4 PE transposes -> one copy to
                dst ([128, 4, 128] view)."""
                ptr = psY.tile([P, 512], bf16, tag="Y", name="ptrq")
                for hh in range(4):
                    nc.tensor.transpose(
                        ptr[:, hh * P:(hh + 1) * P], src_bf[:, hh, :], ident
                    )
                nc.vector.tensor_copy(dst, ptr.rearrange("p (s n) -> p s n", s=4))

            def cos_tiles(cap, sap, t0, name):
                cos4 = cspool.tile([P, 4, 64], f32, tag="cs4", name=f"c{name}")
                sin4 = cspool.tile([P, 4, 64], f32, tag="sn4", name=f"s{name}")
                nc.gpsimd.dma_start(out=cos4, in_=bcast4(cap[t0:t0 + P, :]))
                nc.gpsimd.dma_start(out=sin4, in_=bcast4(sap[t0:t0 + P, :]))
                return cos4, sin4

            # ---------------- phase 0a + 1Q: own-row transposes, Q proj ----
            xoT = [
                bigpool.tile([P, 4, QROWS], bf16, tag=f"xT{gr}", name=f"xoT{gr}")
                for gr in range(4)
            ]
            for tt in range(NQT):
                x_transpose(xo[tt * P:(tt + 1) * P, :], xoT, tt * P, f"o{tt}")

            for s in range(4):
                wsl = load_w_slab(wq, s * 512, f"wq{s}")
                for tt in range(NQT):
                    ps = psS.tile([P, 512], f32, tag="S", name="psq")
                    for kt in range(NCT):
                        nc.tensor.matmul(
                            ps,
                            xoT[kt // 4][:, kt % 4, tt * P:(tt + 1) * P],
                            wsl[:, kt, :],
                            start=(kt == 0),
                            stop=(kt == NCT - 1),
                        )
                    cos4, sin4 = cos_tiles(coso, sino, tt * P, f"q{s}{tt}")
                    qhat = qhpool.tile([P, 4, HD], bf16, tag="qhat", name="qhat")
                    rope_rms(ps, cos4, sin4, qhat, eps_q, 1.0 / HD)
                    drain_pending(1)
                    pending.append(
                        lambda qhat=qhat, s=s, tt=tt: pack_transpose(
                            qhat,
                            qT[:, 4 * s:4 * s + 4, (3 - tt) * P:(4 - tt) * P],
                        )
                    )
            drain_pending()

            # ---------------- phase 0b + 1KV: two halves ----------------
            for half in range(2):
                xfT = [
                    bigpool.tile([P, 4, 8 * P], bf16, tag=f"xT{gr}",
                                 name=f"xfT{half}{gr}")
                    for gr in range(4)
                ]
                for tt in range(8 * half, 8 * half + 8):
                    x_transpose(xf[tt * P:(tt + 1) * P, :], xfT,
                                (tt - 8 * half) * P, f"f{tt}")
                wslk = load_w_slab(wk, 0, f"wk{half}")
                for tt in range(8 * half, 8 * half + 8):
                    tl = tt - 8 * half
                    ps = psS.tile([P, 512], f32, tag="S", name="psk")
                    for kt in range(NCT):
                        nc.tensor.matmul(
                            ps,
                            xfT[kt // 4][:, kt % 4, tl * P:(tl + 1) * P],
                            wslk[:, kt, :],
                            start=(kt == 0),
                            stop=(kt == NCT - 1),
                        )
                    cos4, sin4 = cos_tiles(cosf, sinf, tt * P, f"k{tt}")
                    khat = qhpool.tile([P, 4, HD], bf16, tag="qhat", name="khat")
                    # fold attn scale into k's rms: 1/sqrt(ss + 128*eps)
                    rope_rms(ps, cos4, sin4, khat, eps_k, 1.0)
                    drain_pending(1)
                    pending.append(
                        lambda khat=khat, tt=tt: pack_transpose(
                            khat, kT[:, 0:4, tt * P:(tt + 1) * P]
                        )
                    )
                drain_pending()
                wslv = load_w_slab(wv, 0, f"wv{half}")
                for tt in range(8 * half, 8 * half + 8):
                    tl = tt - 8 * half
                    psv = psS.tile([P, 512], f32, tag="S", name="psv")
                    for kt in range(NCT):
                        nc.tensor.matmul(
                            psv,
                            xfT[kt // 4][:, kt % 4, tl * P:(tl + 1) * P],
                            wslv[:, kt, :],
                            start=(kt == 0),
                            stop=(kt == NCT - 1),
                        )
                    nc.scalar.copy(
                        vA[:, tt, :, :], psv.rearrange("p (h d) -> p h d", h=4)
                    )

            # ---------------- phase 2: attention (scores-transposed) -------
            # Head-major: for each head, one variable-width matmul per
            # key-tile streams all still-valid query slots at once
            # (kt 0-3 -> N=512 ... kt 12-15 -> N=128). yt/den accumulate all
            # four slots per head in single psum tiles.
            tail_state = []  # (yt_psum, den_psum, h)

            def emit_tail():
                if not tail_state:
                    return
                yt, den, h = tail_state.pop(0)
                rinv = smallf.tile([P, QROWS], f32, tag="rq", name="rqinv")
                nc.vector.reciprocal_approx_fast(rinv, den)
                nc.vector.tensor_tensor(yT[:, h, :], yt, rinv, op=OP.mult)

            def emit_dpv(ent, yt, den, kvh):
                ppt, pkt, pn = ent
                last = pkt == NT - 1
                nc.tensor.matmul(
                    den[:, 0:pn], ones128, ppt[:, 0:pn],
                    start=(pkt == 0), stop=last, skip_group_check=True,
                )
                nc.tensor.matmul(
                    yt[:, 0:pn], vA[:, pkt, kvh, :], ppt[:, 0:pn],
                    start=(pkt == 0), stop=last, skip_group_check=True,
                )

            def load_wo_slab(s3):
                w3 = wopool.tile([P, NCT, 512], bf16, tag="wo3", name=f"wo{s3}")
                for gr in range(4):
                    nc.gpsimd.dma_start(
                        out=w3[:, 4 * gr:4 * gr + 4, :],
                        in_=wo[:, s3 * 512:s3 * 512 + 512].rearrange(
                            "(a p) n -> p a n", p=P
                        )[:, 4 * gr:4 * gr + 4, :],
                    )
                return w3

            # prefetch the first two wo slabs; their DMAs run under phase 2
            w3s = {0: load_wo_slab(0), 1: load_wo_slab(1)}

            for h in range(NH):
                kvh = h // (NH // NKV)
                yt = psY.tile([P, QROWS], f32, tag="Y", name="yt")
                den = psD.tile([P, QROWS], f32, tag="D", name="den")
                dq = []  # exp'd tiles awaiting den/PV (2-deep pipeline)
                for kt in range(NT):
                    # q-slot columns are stored high-slot-first, so the
                    # still-valid slots for key tile kt are columns [0, n)
                    n = QROWS - (kt // 4) * P
                    S = psS.tile([P, 512], f32, tag="S", name="Sb")
                    nc.tensor.matmul(
                        S[:, 0:n],
                        kT[:, kvh, kt * P:(kt + 1) * P],
                        qT[:, h, 0:n],
                        start=True,
                        stop=False,
                        skip_group_check=True,
                    )
                    # causal mask for the diagonal slot (last 128 valid cols),
                    # accumulated on the PE via an identity matmul
                    nc.tensor.matmul(
                        S[:, n - P:n], ident, qmask[:, kt // 4, kt % 4, :],
                        start=False, stop=True, skip_group_check=True,
                    )
                    if kt == 0 and tail_state:
                        emit_tail()
                    # attn scale already folded into k's rms normalization
                    pt = ptpool.tile([P, 512], bf16, tag="pt", name="pt")
                    nc.scalar.activation(pt[:, 0:n], S[:, 0:n], AF.Exp, scale=1.0)
                    dq.append((pt, kt, n))
                    if len(dq) > 2:
                        emit_dpv(dq.pop(0), yt, den, kvh)
                while dq:
                    emit_dpv(dq.pop(0), yt, den, kvh)
                tail_state.append((yt, den, h))
            emit_tail()

            # ---------------- phase 3: output projection ----------------
            for s3 in range(4):
                w3 = w3s.pop(s3)
                if s3 + 2 < 4:
                    w3s[s3 + 2] = load_wo_slab(s3 + 2)
                for qt in range(4):
                    ps = psS.tile([P, 512], f32, tag="S", name="ps3")
                    for ct in range(NCT):
                        nc.tensor.matmul(
                            ps,
                            yT[:, ct, (3 - qt) * P:(4 - qt) * P],
                            w3[:, ct, :],
                            start=(ct == 0),
                            stop=(ct == NCT - 1),
                        )
                    ot = outpool.tile([P, 512], f32, tag="ot", name="ot")
                    nc.vector.tensor_copy(ot, ps)
                    nc.sync.dma_start(
                        out=yo[qt * P:(qt + 1) * P, s3 * 512:(s3 + 1) * 512],
                        in_=ot,
                    )

    nc.compile()
    return nc


def _get_nc():
    if "nc" not in _CACHE:
        _CACHE["nc"] = _build()
    return _CACHE["nc"]


def _in_maps(x, cosr, sinr, wq, wk, wv, wo):
    maps = []
    for core in range(8):
        b, g = core // 4, core % 4
        rows = _rows(g)
        maps.append({
            "xf": np.ascontiguousarray(x[b]),
            "xo": np.ascontiguousarray(x[b][rows]),
            "cosf": cosr,
            "sinf": sinr,
            "coso": np.ascontiguousarray(cosr[rows]),
            "sino": np.ascontiguousarray(sinr[rows]),
            "wq": wq, "wk": wk, "wv": wv, "wo": wo,
            "qm": _qmask_t(g),
        })
    return maps


def kernel(x, cos, sin, wq, wk, wv, wo):
    from concourse.bass_utils import run_bass_kernel_spmd

    x = np.ascontiguousarray(np.asarray(x, np.float32))
    cosr = np.ascontiguousarray(np.asarray(cos, np.float32).reshape(T, HD // 2))
    sinr = np.ascontiguousarray(np.asarray(sin, np.float32).reshape(T, HD // 2))
    wq = np.ascontiguousarray(np.asarray(wq, np.float32))
    wk = np.ascontiguousarray(np.asarray(wk, np.float32))
    wv = np.ascontiguousarray(np.asarray(wv, np.float32))
    wo = np.ascontiguousarray(np.asarray(wo, np.float32))

    nc = _get_nc()
    maps = _in_maps(x, cosr, sinr, wq, wk, wv, wo)
    _CACHE["in_maps"] = maps
    res = run_bass_kernel_spmd(nc, maps, list(range(8)))
    y = np.empty((B, T, C), np.float32)
    for core in range(8):
        b, g = core // 4, core % 4
        y[b][_rows(g)] = res.results[core]["yo"]
    return y



# revision 6
# speedup vs baseline: 1.1891x; 1.1891x over previous
"""Causal self-attention (GQA, rope, qk-rmsnorm) on 8 TRN2 NeuronCores.

Sharding: core = (b, g), b = core // 4 (batch), g = core % 4.
Each core owns 8 interleaved 64-row query slots of its batch (balanced
causal assignment), computes Q for those 512 rows (all 16 heads), K/V
for kv-head g only (sharded 4-way), all-gathers K/V within its 4-core
batch group, runs attention for all heads over its own queries, and
projects its 512-row output slice locally (no output collective).

All PE-facing tensors stay transposed ([feature, token]); rope's
cross-partition half-swap runs on the PE via two constant combine
matrices, rms-norm partition sums via a ones matmul, and the causal
mask is a post-exp 0/1 multiply on the vector engine. The host
pre-transposes x and converts weights to bf16.
"""

import sys

if "/opt/trn_rl_repo" not in sys.path:
    sys.path.insert(0, "/opt/trn_rl_repo")

import numpy as np
import ml_dtypes

BF16 = ml_dtypes.bfloat16

B, T, C = 2, 2048, 2048
NH, NKV = 16, 4
HD = 128
P = 128
NCT = C // P           # 16 contraction tiles
QR = 512               # own query rows per core
NKT = T // P           # 16 key tiles
A_SHAPE = [16, 14, 12, 10, 8, 6, 4, 2]  # uniform key-tile count per slot
EPS = float(np.finfo(np.float32).eps)

_CACHE = {}


def _slots64(g):
    """Own 64-row query chunks, descending. Balanced: sum of causal
    key-needs is equal across g."""
    return [31 - g, 24 + g, 23 - g, 16 + g, 15 - g, 8 + g, 7 - g, g]


def _qrows(g):
    return np.concatenate([np.arange(c * 64, (c + 1) * 64) for c in _slots64(g)])


def _mask01(g):
    """0/1 mask, applied to exp'd score tiles: for slot i the program
    masks key tiles A_SHAPE[i]-2 and A_SHAPE[i]-1 (j = 0, 1); entry is
    1 where key <= query."""
    m = np.zeros((8, 2, P, 64), np.float32)
    sl = _slots64(g)
    for i in range(8):
        q = sl[i] * 64 + np.arange(64)[None, :]
        for j in range(2):
            kt = A_SHAPE[i] - 2 + j
            k = kt * P + np.arange(P)[:, None]
            m[i, j] = (k <= q).astype(np.float32)
    return np.ascontiguousarray(m.astype(BF16))


def _rope_mats():
    """ro = A @ m1 + B @ m2 with m1 = q*[cos;sin], m2 = q*[sin;cos]:
    ro[0:64] = m1[0:64] + m1[64:128]; ro[64:128] = m2[64:128] - m2[0:64].
    Returned transposed ([contraction, out_partition]) for use as lhsT."""
    Am = np.zeros((P, P), np.float32)
    Bm = np.zeros((P, P), np.float32)
    for o in range(64):
        Am[o, o] = 1.0
        Am[o + 64, o] = 1.0
    for o in range(64, 128):
        Bm[o, o] = 1.0
        Bm[o - 64, o] = -1.0
    return np.ascontiguousarray(Am.astype(BF16)), np.ascontiguousarray(Bm.astype(BF16))


def _build():
    import concourse.bacc as bacc
    import concourse.mybir as mybir
    import concourse.tile as tile

    f32 = mybir.dt.float32
    bf16 = mybir.dt.bfloat16
    AF = mybir.ActivationFunctionType
    OP = mybir.AluOpType

    nc = bacc.Bacc("TRN2", target_bir_lowering=False, debug=False, num_devices=8)

    xoT = nc.dram_tensor("xoT", [C, QR], bf16, kind="ExternalInput").ap()
    xfT = nc.dram_tensor("xfT", [C, T], bf16, kind="ExternalInput").ap()
    cso = nc.dram_tensor("cso", [P, QR], f32, kind="ExternalInput").ap()
    sno = nc.dram_tensor("sno", [P, QR], f32, kind="ExternalInput").ap()
    csf = nc.dram_tensor("csf", [P, T], f32, kind="ExternalInput").ap()
    snf = nc.dram_tensor("snf", [P, T], f32, kind="ExternalInput").ap()
    wq = nc.dram_tensor("wq", [C, C], bf16, kind="ExternalInput").ap()
    wks = nc.dram_tensor("wks", [C, HD], bf16, kind="ExternalInput").ap()
    wvs = nc.dram_tensor("wvs", [C, HD], bf16, kind="ExternalInput").ap()
    wo = nc.dram_tensor("wo", [C, C], bf16, kind="ExternalInput").ap()
    msk = nc.dram_tensor("msk", [8, 2, P, 64], bf16, kind="ExternalInput").ap()
    rpA = nc.dram_tensor("rpA", [P, P], bf16, kind="ExternalInput").ap()
    rpB = nc.dram_tensor("rpB", [P, P], bf16, kind="ExternalInput").ap()
    yo = nc.dram_tensor("yo", [QR, C], f32, kind="ExternalOutput").ap()

    with tile.TileContext(nc) as tc:
        with (
            tc.tile_pool(name="singles", bufs=1) as singles,
            tc.tile_pool(name="big", bufs=1) as bigpool,
            tc.tile_pool(name="xc", bufs=2) as xcpool,
            tc.tile_pool(name="wqh", bufs=3) as wqpool,
            tc.tile_pool(name="rope", bufs=2) as ropep,
            tc.tile_pool(name="stage", bufs=2) as stagep,
            tc.tile_pool(name="pt", bufs=4) as ptpool,
            tc.tile_pool(name="small", bufs=2) as smallp,
            tc.tile_pool(name="outs", bufs=2) as outpool,
            tc.tile_pool(name="psS", bufs=3, space="PSUM") as psS,
            tc.tile_pool(name="psY", bufs=2, space="PSUM") as psY,
            tc.tile_pool(name="psD", bufs=2, space="PSUM") as psD,
            tc.tile_pool(name="dram", bufs=1, space="DRAM") as drampool,
        ):
            ones128 = singles.tile([P, P], bf16)
            nc.vector.memset(ones128, 1.0)
            eps_q = singles.tile([P, 1], f32)
            nc.vector.memset(eps_q, HD * EPS)
            eps_k = singles.tile([P, 1], f32)
            nc.vector.memset(eps_k, EPS)
            rpA_sb = singles.tile([P, P], bf16)
            nc.sync.dma_start(out=rpA_sb, in_=rpA)
            rpB_sb = singles.tile([P, P], bf16)
            nc.sync.dma_start(out=rpB_sb, in_=rpB)
            csf_sb = singles.tile([P, T], f32)
            nc.sync.dma_start(out=csf_sb, in_=csf)
            snf_sb = singles.tile([P, T], f32)
            nc.sync.dma_start(out=snf_sb, in_=snf)
            cso_sb = singles.tile([P, QR], f32)
            nc.sync.dma_start(out=cso_sb, in_=cso)
            sno_sb = singles.tile([P, QR], f32)
            nc.sync.dma_start(out=sno_sb, in_=sno)
            mk = singles.tile([P, 8, 2, 64], bf16)
            nc.sync.dma_start(out=mk, in_=msk.rearrange("i j p n -> p i j n"))

            kv_in = drampool.tile([P, 2 * T], bf16, tag="kvin")
            kv_out = drampool.tile([NKV * P, 2 * T], bf16, tag="kvout")

            def rope_rms(ps, cs_ap, sn_ap, out_ap, scale, bias_ap, nm):
                """ps: [128, 512] psum f32 = projected [head_dim, tok].
                Applies rope (PE combine) + rms-norm, writes bf16 out_ap."""
                m1 = ropep.tile([P, QR], bf16, tag="m1", name=f"m1{nm}")
                m2 = ropep.tile([P, QR], bf16, tag="m2", name=f"m2{nm}")
                nc.vector.tensor_tensor(m1, ps, cs_ap, op=OP.mult)
                nc.vector.tensor_tensor(m2, ps, sn_ap, op=OP.mult)
                ro = psY.tile([P, QR], f32, tag="Y", name=f"ro{nm}")
                nc.tensor.matmul(ro, rpA_sb, m1, start=True, stop=False)
                nc.tensor.matmul(ro, rpB_sb, m2, start=False, stop=True)
                sq = ropep.tile([P, QR], bf16, tag="sq", name=f"sq{nm}")
                nc.scalar.activation(sq, ro, AF.Square)
                ss = psD.tile([P, QR], f32, tag="D", name=f"ss{nm}")
                nc.tensor.matmul(ss, ones128, sq, start=True, stop=True)
                rms = ropep.tile([P, QR], f32, tag="rms", name=f"rms{nm}")
                nc.scalar.activation(rms, ss, AF.Sqrt, bias=bias_ap, scale=scale)
                rinv = ropep.tile([P, QR], f32, tag="rinv", name=f"rinv{nm}")
                nc.vector.reciprocal_approx_fast(rinv, rms)
                nc.vector.tensor_tensor(out_ap, ro, rinv, op=OP.mult)

            # ---------------- phase 1: K/V projection (own kv head) -----
            wks_sb = singles.tile([P, NCT, HD], bf16)
            nc.sync.dma_start(out=wks_sb, in_=wks.rearrange("(a p) n -> p a n", p=P))
            wvs_sb = singles.tile([P, NCT, HD], bf16)
            nc.sync.dma_start(out=wvs_sb, in_=wvs.rearrange("(a p) n -> p a n", p=P))

            for j in range(4):
                xc = xcpool.tile([P, NCT, QR], bf16, tag="xc", name=f"xc{j}")
                for hf in range(2):
                    nc.sync.dma_start(
                        out=xc[:, 8 * hf:8 * hf + 8, :],
                        in_=xfT[:, j * QR:(j + 1) * QR].rearrange(
                            "(a p) n -> p a n", p=P
                        )[:, 8 * hf:8 * hf + 8, :],
                    )
                kp = psS.tile([P, QR], f32, tag="S", name=f"kp{j}")
                for ct in range(NCT):
                    nc.tensor.matmul(
                        kp, wks_sb[:, ct, :], xc[:, ct, :],
                        start=(ct == 0), stop=(ct == NCT - 1),
                    )
                kst = stagep.tile([P, QR], bf16, tag="kst", name=f"kst{j}")
                rope_rms(
                    kp, csf_sb[:, j * QR:(j + 1) * QR],
                    snf_sb[:, j * QR:(j + 1) * QR], kst, 1.0 / HD, eps_k, f"k{j}",
                )
                nc.sync.dma_start(out=kv_in[:, j * QR:(j + 1) * QR], in_=kst)

                vp = psS.tile([P, QR], f32, tag="S", name=f"vp{j}")
                for ct in range(NCT):
                    nc.tensor.matmul(
                        vp, wvs_sb[:, ct, :], xc[:, ct, :],
                        start=(ct == 0), stop=(ct == NCT - 1),
                    )
                vtmp = stagep.tile([P, QR], bf16, tag="vtmp", name=f"vtmp{j}")
                nc.scalar.copy(vtmp, vp)
                vstage = stagep.tile([P, 4, P], bf16, tag="vst", name=f"vst{j}")
                for tk in range(4):
                    nc.sync.dma_start_transpose(
                        vstage[:, tk, :], vtmp[:, tk * P:(tk + 1) * P]
                    )
                nc.sync.dma_start(
                    out=kv_in[:, T + j * QR:T + (j + 1) * QR], in_=vstage
                )

            nc.gpsimd.collective_compute(
                "AllGather",
                OP.bypass,
                replica_groups=[[0, 1, 2, 3], [4, 5, 6, 7]],
                ins=[kv_in.opt()],
                outs=[kv_out.opt()],
            )

            # ---------------- phase 2: Q projection (all heads) ---------
            # shares the xc pool (xc chunks are dead by now)
            xo_sb = xcpool.tile([P, NCT, QR], bf16, tag="xc", name="xo")
            for hf in range(4):
                nc.sync.dma_start(
                    out=xo_sb[:, 4 * hf:4 * hf + 4, :],
                    in_=xoT.rearrange("(a p) n -> p a n", p=P)[:, 4 * hf:4 * hf + 4, :],
                )
            qT = bigpool.tile([P, NH, QR], bf16, tag="qT")
            for h in range(NH):
                wqh = wqpool.tile([P, NCT, HD], bf16, tag="wqh", name=f"wqh{h}")
                nc.scalar.dma_start(
                    out=wqh,
                    in_=wq[:, h * HD:(h + 1) * HD].rearrange("(a p) n -> p a n", p=P),
                )
                qp = psS.tile([P, QR], f32, tag="S", name=f"qp{h}")
                for ct in range(NCT):
                    nc.tensor.matmul(
                        qp, wqh[:, ct, :], xo_sb[:, ct, :],
                        start=(ct == 0), stop=(ct == NCT - 1),
                    )
                rope_rms(qp, cso_sb, sno_sb, qT[:, h, :], 1.0, eps_q, f"q{h}")

            # K/V gathered from the group (depends on the collective)
            kT = bigpool.tile([P, NKV, T], bf16, tag="kT")
            vA = bigpool.tile([P, NKT, NKV, HD], bf16, tag="vA")
            for kv in range(NKV):
                nc.gpsimd.dma_start(
                    out=kT[:, kv, :], in_=kv_out[kv * P:(kv + 1) * P, 0:T]
                )
                nc.gpsimd.dma_start(
                    out=vA[:, :, kv, :],
                    in_=kv_out[kv * P:(kv + 1) * P, T:2 * T].rearrange(
                        "p (t d) -> p t d", d=HD
                    ),
                )

            # ---------------- phase 3: attention ------------------------
            yT = bigpool.tile([P, NH, QR], bf16, tag="yT")

            def load_wo_slab(s):
                # shares the xc pool; slab s=1 evicts xo_sb (read-dep safe)
                w3 = xcpool.tile([P, NCT, QR], bf16, tag="xc", name=f"wo{s}")
                for gr in range(4):
                    nc.gpsimd.dma_start(
                        out=w3[:, 4 * gr:4 * gr + 4, :],
                        in_=wo[:, s * QR:(s + 1) * QR].rearrange(
                            "(a p) n -> p a n", p=P
                        )[:, 4 * gr:4 * gr + 4, :],
                    )
                return w3

            w3s = {0: load_wo_slab(0), 1: load_wo_slab(1)}

            tail = []

            def emit_tail():
                yt, den, h = tail.pop(0)
                rinv = smallp.tile([P, QR], f32, tag="rq", name=f"rq{h}")
                nc.vector.reciprocal_approx_fast(rinv, den)
                nc.vector.tensor_tensor(yT[:, h, :], yt, rinv, op=OP.mult)

            def emit_dpv(ent, yt, den, kv):
                pt, kt, n = ent
                last = kt == NKT - 1
                nc.tensor.matmul(
                    den[:, 0:n], ones128, pt[:, 0:n],
                    start=(kt == 0), stop=last, skip_group_check=True,
                )
                nc.tensor.matmul(
                    yt[:, 0:n], vA[:, kt, kv, :], pt[:, 0:n],
                    start=(kt == 0), stop=last, skip_group_check=True,
                )

            for h in range(NH):
                kv = h // (NH // NKV)
                yt = psY.tile([P, QR], f32, tag="Y", name=f"yt{h}")
                den = psD.tile([P, QR], f32, tag="D", name=f"den{h}")
                dq = []
                for kt in range(NKT):
                    n = 64 * (8 - kt // 2)
                    S = psS.tile([P, QR], f32, tag="S", name=f"S{h}_{kt}")
                    nc.tensor.matmul(
                        S[:, 0:n],
                        kT[:, kv, kt * P:(kt + 1) * P],
                        qT[:, h, 0:n],
                        start=True, stop=True, skip_group_check=True,
                    )
                    if kt == 0 and tail:
                        emit_tail()
                    pt = ptpool.tile([P, QR], bf16, tag="pt", name=f"pt{h}_{kt}")
                    nc.scalar.activation(pt[:, 0:n], S[:, 0:n], AF.Exp, scale=1.0)
                    # exactly one slot is in the mask window for each kt
                    i = (15 - kt) // 2
                    nc.vector.tensor_tensor(
                        pt[:, n - 64:n], pt[:, n - 64:n], mk[:, i, kt % 2, :],
                        op=OP.mult,
                    )
                    dq.append((pt, kt, n))
                    if len(dq) > 2:
                        emit_dpv(dq.pop(0), yt, den, kv)
                while dq:
                    emit_dpv(dq.pop(0), yt, den, kv)
                tail.append((yt, den, h))
            emit_tail()

            # ---------------- phase 4: output projection ----------------
            for s in range(4):
                w3 = w3s.pop(s)
                if s + 2 < 4:
                    w3s[s + 2] = load_wo_slab(s + 2)
                for rt in range(4):
                    ps = psS.tile([P, QR], f32, tag="S", name=f"o{s}_{rt}")
                    for h in range(NH):
                        nc.tensor.matmul(
                            ps, yT[:, h, rt * P:(rt + 1) * P], w3[:, h, :],
                            start=(h == 0), stop=(h == NH - 1),
                        )
                    ot = outpool.tile([P, QR], f32, tag="ot", name=f"ot{s}_{rt}")
                    nc.vector.tensor_copy(ot, ps)
                    nc.sync.dma_start(
                        out=yo[rt * P:(rt + 1) * P, s * QR:(s + 1) * QR], in_=ot
                    )

    nc.compile()
    return nc


def _get_nc():
    if "nc" not in _CACHE:
        _CACHE["nc"] = _build()
    return _CACHE["nc"]


def _prep_in_maps(x, cos, sin, wq, wk, wv, wo):
    x = np.asarray(x, np.float32)
    cosr = np.asarray(cos, np.float32).reshape(T, HD // 2)
    sinr = np.asarray(sin, np.float32).reshape(T, HD // 2)
    wqb = np.ascontiguousarray(np.asarray(wq, np.float32).astype(BF16))
    wob = np.ascontiguousarray(np.asarray(wo, np.float32).astype(BF16))
    wkf = np.asarray(wk, np.float32)
    wvf = np.asarray(wv, np.float32)

    csf = np.ascontiguousarray(np.concatenate([cosr.T, sinr.T], axis=0))
    snf = np.ascontiguousarray(np.concatenate([sinr.T, cosr.T], axis=0))
    rpA_np, rpB_np = _rope_mats()

    maps = []
    for core in range(8):
        b, g = core // 4, core % 4
        qr = _qrows(g)
        xT_b = x[b].T
        maps.append({
            "xoT": np.ascontiguousarray(xT_b[:, qr].astype(BF16)),
            "xfT": np.ascontiguousarray(xT_b.astype(BF16)),
            "cso": np.ascontiguousarray(csf[:, qr]),
            "sno": np.ascontiguousarray(snf[:, qr]),
            "csf": csf,
            "snf": snf,
            "wq": wqb,
            "wks": np.ascontiguousarray(wkf[:, g * HD:(g + 1) * HD].astype(BF16)),
            "wvs": np.ascontiguousarray(wvf[:, g * HD:(g + 1) * HD].astype(BF16)),
            "wo": wob,
            "msk": _mask01(g),
            "rpA": rpA_np,
            "rpB": rpB_np,
        })
    return maps


def kernel(x, cos, sin, wq, wk, wv, wo):
    from concourse.bass_utils import run_bass_kernel_spmd

    nc = _get_nc()
    maps = _prep_in_maps(x, cos, sin, wq, wk, wv, wo)
    _CACHE["in_maps"] = maps
    res = run_bass_kernel_spmd(nc, maps, list(range(8)))
    y = np.empty((B, T, C), np.float32)
    for core in range(8):
        b, g = core // 4, core % 4
        y[b][_qrows(g)] = res.results[core]["yo"]
    return y


# revision 8
# speedup vs baseline: 1.2097x; 1.0174x over previous
"""Causal self-attention (GQA, rope, qk-rmsnorm) on 8 TRN2 NeuronCores.

Sharding: core = (b, g), b = core // 4 (batch), g = core % 4.
Each core owns 8 interleaved 64-row query slots of its batch (balanced
causal assignment), computes Q for those 512 rows (all 16 heads), K/V
for kv-head g only (sharded 4-way), all-gathers K/V within its 4-core
batch group (overlapped with the Q projection), runs attention for all
heads over its own queries, and projects its 512-row output slice
locally (no output collective).

All PE-facing tensors stay transposed ([feature, token]); rope's
cross-partition half-swap runs on the PE via two constant combine
matrices, rms-norm partition sums via a ones matmul, and the causal
mask is a post-exp 0/1 multiply on the vector engine. The host
pre-transposes x and converts weights to bf16.

Engine queues: sync = pure loads, scalar = activation compute (+ the
v DMA-transposes), gpsimd = dependent stores, the collective, gather-
ins, and the den pair-adds.
"""

import sys

if "/opt/trn_rl_repo" not in sys.path:
    sys.path.insert(0, "/opt/trn_rl_repo")

import numpy as np
import ml_dtypes

BF16 = ml_dtypes.bfloat16

B, T, C = 2, 2048, 2048
NH, NKV = 16, 4
HD = 128
P = 128
NCT = C // P           # 16 contraction tiles
QR = 512               # own query rows per core
NKT = T // P           # 16 key tiles
A_SHAPE = [16, 14, 12, 10, 8, 6, 4, 2]  # uniform key-tile count per slot
EPS = float(np.finfo(np.float32).eps)
USE_DSQRT = False      # Dsqrt has no ACT table on this stack; use Sqrt+recip

_CACHE = {}


def _slots64(g):
    """Own 64-row query chunks, descending. Balanced: sum of causal
    key-needs is equal across g."""
    return [31 - g, 24 + g, 23 - g, 16 + g, 15 - g, 8 + g, 7 - g, g]


def _qrows(g):
    return np.concatenate([np.arange(c * 64, (c + 1) * 64) for c in _slots64(g)])


def _mask01(g):
    """0/1 mask, applied to exp'd score tiles: for slot i the program
    masks key tiles A_SHAPE[i]-2 and A_SHAPE[i]-1 (j = 0, 1); entry is
    1 where key <= query."""
    m = np.zeros((8, 2, P, 64), np.float32)
    sl = _slots64(g)
    for i in range(8):
        q = sl[i] * 64 + np.arange(64)[None, :]
        for j in range(2):
            kt = A_SHAPE[i] - 2 + j
            k = kt * P + np.arange(P)[:, None]
            m[i, j] = (k <= q).astype(np.float32)
    return np.ascontiguousarray(m.astype(BF16))


def _rope_mats():
    """ro = A @ m1 + B @ m2 with m1 = q*[cos;sin], m2 = q*[sin;cos]:
    ro[0:64] = m1[0:64] + m1[64:128]; ro[64:128] = m2[64:128] - m2[0:64].
    Returned transposed ([contraction, out_partition]) for use as lhsT."""
    Am = np.zeros((P, P), np.float32)
    Bm = np.zeros((P, P), np.float32)
    for o in range(64):
        Am[o, o] = 1.0
        Am[o + 64, o] = 1.0
    for o in range(64, 128):
        Bm[o, o] = 1.0
        Bm[o - 64, o] = -1.0
    return np.ascontiguousarray(Am.astype(BF16)), np.ascontiguousarray(Bm.astype(BF16))


def _build():
    import concourse.bacc as bacc
    import concourse.mybir as mybir
    import concourse.tile as tile

    f32 = mybir.dt.float32
    bf16 = mybir.dt.bfloat16
    AF = mybir.ActivationFunctionType
    OP = mybir.AluOpType

    nc = bacc.Bacc("TRN2", target_bir_lowering=False, debug=False, num_devices=8)

    xoT = nc.dram_tensor("xoT", [C, QR], bf16, kind="ExternalInput").ap()
    xfT = nc.dram_tensor("xfT", [C, T], bf16, kind="ExternalInput").ap()
    cso = nc.dram_tensor("cso", [P, QR], f32, kind="ExternalInput").ap()
    sno = nc.dram_tensor("sno", [P, QR], f32, kind="ExternalInput").ap()
    csf = nc.dram_tensor("csf", [P, T], f32, kind="ExternalInput").ap()
    snf = nc.dram_tensor("snf", [P, T], f32, kind="ExternalInput").ap()
    wq = nc.dram_tensor("wq", [C, C], bf16, kind="ExternalInput").ap()
    wks = nc.dram_tensor("wks", [C, HD], bf16, kind="ExternalInput").ap()
    wvs = nc.dram_tensor("wvs", [C, HD], bf16, kind="ExternalInput").ap()
    wo = nc.dram_tensor("wo", [C, C], bf16, kind="ExternalInput").ap()
    msk = nc.dram_tensor("msk", [8, 2, P, 64], bf16, kind="ExternalInput").ap()
    rpA = nc.dram_tensor("rpA", [P, P], bf16, kind="ExternalInput").ap()
    rpB = nc.dram_tensor("rpB", [P, P], bf16, kind="ExternalInput").ap()
    yo = nc.dram_tensor("yo", [QR, C], f32, kind="ExternalOutput").ap()

    with tile.TileContext(nc) as tc:
        with (
            tc.tile_pool(name="singles", bufs=1) as singles,
            tc.tile_pool(name="big", bufs=1) as bigpool,
            tc.tile_pool(name="xc", bufs=2) as xcpool,
            tc.tile_pool(name="wqh", bufs=3) as wqpool,
            tc.tile_pool(name="rope", bufs=3) as ropep,
            tc.tile_pool(name="stage", bufs=2) as stagep,
            tc.tile_pool(name="pt", bufs=6) as ptpool,
            tc.tile_pool(name="padd", bufs=3) as paddp,
            tc.tile_pool(name="small", bufs=2) as smallp,
            tc.tile_pool(name="outs", bufs=2) as outpool,
            tc.tile_pool(name="psS", bufs=4, space="PSUM") as psS,
            tc.tile_pool(name="psY", bufs=2, space="PSUM") as psY,
            tc.tile_pool(name="psD", bufs=2, space="PSUM") as psD,
            tc.tile_pool(name="dram", bufs=1, space="DRAM") as drampool,
        ):
            ones128 = singles.tile([P, P], bf16)
            nc.vector.memset(ones128, 1.0)
            eps_q = singles.tile([P, 1], f32)
            nc.vector.memset(eps_q, HD * EPS / 4.0 if USE_DSQRT else HD * EPS)
            eps_k = singles.tile([P, 1], f32)
            nc.vector.memset(eps_k, EPS / 4.0 if USE_DSQRT else EPS)
            sc_q = 0.25 if USE_DSQRT else 1.0
            sc_k = (1.0 / (4.0 * HD)) if USE_DSQRT else 1.0 / HD
            rpA_sb = singles.tile([P, P], bf16)
            nc.sync.dma_start(out=rpA_sb, in_=rpA)
            rpB_sb = singles.tile([P, P], bf16)
            nc.sync.dma_start(out=rpB_sb, in_=rpB)
            csf_sb = singles.tile([P, T], f32)
            nc.sync.dma_start(out=csf_sb, in_=csf)
            snf_sb = singles.tile([P, T], f32)
            nc.sync.dma_start(out=snf_sb, in_=snf)
            cso_sb = singles.tile([P, QR], f32)
            nc.sync.dma_start(out=cso_sb, in_=cso)
            sno_sb = singles.tile([P, QR], f32)
            nc.sync.dma_start(out=sno_sb, in_=sno)
            mk = singles.tile([P, 8, 2, 64], bf16)
            nc.sync.dma_start(out=mk, in_=msk.rearrange("i j p n -> p i j n"))

            kv_in = drampool.tile([P, 2 * T], bf16, tag="kvin")
            kv_out = drampool.tile([NKV * P, 2 * T], bf16, tag="kvout")

            def rope_rms(ps, cs_ap, sn_ap, out_ap, scale, bias_ap, nm):
                """ps: [128, 512] psum f32 = projected [head_dim, tok].
                Applies rope (PE combine) + rms-norm, writes bf16 out_ap."""
                m1 = ropep.tile([P, QR], bf16, tag="m1", name=f"m1{nm}")
                m2 = ropep.tile([P, QR], bf16, tag="m2", name=f"m2{nm}")
                nc.vector.tensor_tensor(m1, ps, cs_ap, op=OP.mult)
                nc.vector.tensor_tensor(m2, ps, sn_ap, op=OP.mult)
                ro = psY.tile([P, QR], f32, tag="Y", name=f"ro{nm}")
                nc.tensor.matmul(ro, rpA_sb, m1, start=True, stop=False)
                nc.tensor.matmul(ro, rpB_sb, m2, start=False, stop=True)
                sq = ropep.tile([P, QR], bf16, tag="sq", name=f"sq{nm}")
                nc.scalar.activation(sq, ro, AF.Square)
                ss = psD.tile([P, QR], f32, tag="D", name=f"ss{nm}")
                nc.tensor.matmul(ss, ones128, sq, start=True, stop=True)
                rinv = ropep.tile([P, QR], f32, tag="rinv", name=f"rinv{nm}")
                if USE_DSQRT:
                    nc.scalar.activation(rinv, ss, AF.Dsqrt, bias=bias_ap, scale=scale)
                else:
                    rms = ropep.tile([P, QR], f32, tag="rms", name=f"rms{nm}")
                    nc.scalar.activation(rms, ss, AF.Sqrt, bias=bias_ap, scale=scale)
                    nc.vector.reciprocal_approx_fast(rinv, rms)
                nc.vector.tensor_tensor(out_ap, ro, rinv, op=OP.mult)

            # ---------------- phase 1: K/V projection (own kv head) -----
            wks_sb = singles.tile([P, NCT, HD], bf16)
            nc.sync.dma_start(out=wks_sb, in_=wks.rearrange("(a p) n -> p a n", p=P))
            wvs_sb = singles.tile([P, NCT, HD], bf16)
            nc.sync.dma_start(out=wvs_sb, in_=wvs.rearrange("(a p) n -> p a n", p=P))

            for j in range(4):
                xc = xcpool.tile([P, NCT, QR], bf16, tag="xc", name=f"xc{j}")
                for hf in range(2):
                    nc.sync.dma_start(
                        out=xc[:, 8 * hf:8 * hf + 8, :],
                        in_=xfT[:, j * QR:(j + 1) * QR].rearrange(
                            "(a p) n -> p a n", p=P
                        )[:, 8 * hf:8 * hf + 8, :],
                    )
                kp = psS.tile([P, QR], f32, tag="S", name=f"kp{j}")
                for ct in range(NCT):
                    nc.tensor.matmul(
                        kp, wks_sb[:, ct, :], xc[:, ct, :],
                        start=(ct == 0), stop=(ct == NCT - 1),
                    )
                kst = stagep.tile([P, QR], bf16, tag="kst", name=f"kst{j}")
                rope_rms(
                    kp, csf_sb[:, j * QR:(j + 1) * QR],
                    snf_sb[:, j * QR:(j + 1) * QR], kst, sc_k, eps_k, f"k{j}",
                )
                nc.gpsimd.dma_start(out=kv_in[:, j * QR:(j + 1) * QR], in_=kst)

                vp = psS.tile([P, QR], f32, tag="S", name=f"vp{j}")
                for ct in range(NCT):
                    nc.tensor.matmul(
                        vp, wvs_sb[:, ct, :], xc[:, ct, :],
                        start=(ct == 0), stop=(ct == NCT - 1),
                    )
                vtmp = stagep.tile([P, QR], bf16, tag="vtmp", name=f"vtmp{j}")
                nc.vector.tensor_copy(vtmp, vp)
                vstage = stagep.tile([P, 4, P], bf16, tag="vst", name=f"vst{j}")
                for tk in range(4):
                    nc.scalar.dma_start_transpose(
                        vstage[:, tk, :], vtmp[:, tk * P:(tk + 1) * P]
                    )
                nc.gpsimd.dma_start(
                    out=kv_in[:, T + j * QR:T + (j + 1) * QR], in_=vstage
                )

            nc.gpsimd.collective_compute(
                "AllGather",
                OP.bypass,
                replica_groups=[[0, 1, 2, 3], [4, 5, 6, 7]],
                ins=[kv_in.opt()],
                outs=[kv_out.opt()],
            )

            # K/V gathered from the group (gpsimd: right after the trigger)
            kT = bigpool.tile([P, NKV, T], bf16, tag="kT")
            vA = bigpool.tile([P, NKT, NKV, HD], bf16, tag="vA")
            for kv in range(NKV):
                nc.gpsimd.dma_start(
                    out=kT[:, kv, :], in_=kv_out[kv * P:(kv + 1) * P, 0:T]
                )
                nc.gpsimd.dma_start(
                    out=vA[:, :, kv, :],
                    in_=kv_out[kv * P:(kv + 1) * P, T:2 * T].rearrange(
                        "p (t d) -> p t d", d=HD
                    ),
                )

            # ---------------- phase 2: Q projection (all heads) ---------
            # shares the xc pool (xc chunks are dead by now)
            xo_sb = xcpool.tile([P, NCT, QR], bf16, tag="xc", name="xo")
            for hf in range(4):
                nc.sync.dma_start(
                    out=xo_sb[:, 4 * hf:4 * hf + 4, :],
                    in_=xoT.rearrange("(a p) n -> p a n", p=P)[:, 4 * hf:4 * hf + 4, :],
                )
            qT = bigpool.tile([P, NH, QR], bf16, tag="qT")
            for h in range(NH):
                wqh = wqpool.tile([P, NCT, HD], bf16, tag="wqh", name=f"wqh{h}")
                nc.sync.dma_start(
                    out=wqh,
                    in_=wq[:, h * HD:(h + 1) * HD].rearrange("(a p) n -> p a n", p=P),
                )
                qp = psS.tile([P, QR], f32, tag="S", name=f"qp{h}")
                for ct in range(NCT):
                    nc.tensor.matmul(
                        qp, wqh[:, ct, :], xo_sb[:, ct, :],
                        start=(ct == 0), stop=(ct == NCT - 1),
                    )
                rope_rms(qp, cso_sb, sno_sb, qT[:, h, :], sc_q, eps_q, f"q{h}")

            # ---------------- phase 3: attention ------------------------
            yT = bigpool.tile([P, NH, QR], bf16, tag="yT")

            def load_wo_slab(s):
                # shares the xc pool; slab s=1 evicts xo_sb (read-dep safe)
                w3 = xcpool.tile([P, NCT, QR], bf16, tag="xc", name=f"wo{s}")
                for gr in range(4):
                    nc.sync.dma_start(
                        out=w3[:, 4 * gr:4 * gr + 4, :],
                        in_=wo[:, s * QR:(s + 1) * QR].rearrange(
                            "(a p) n -> p a n", p=P
                        )[:, 4 * gr:4 * gr + 4, :],
                    )
                return w3

            w3s = {0: load_wo_slab(0), 1: load_wo_slab(1)}

            tail = []

            def emit_tail():
                yt, den, h = tail.pop(0)
                rinv = smallp.tile([P, QR], f32, tag="rq", name=f"rq{h}")
                nc.vector.reciprocal_approx_fast(rinv, den)
                nc.vector.tensor_tensor(yT[:, h, :], yt, rinv, op=OP.mult)

            def emit_pair(ent, yt, den, kv):
                """ent = (m, n, pa, pb) with pa/pb the exp'd tiles for
                kt = 2m, 2m+1 (pb is a column-slice view for packed
                pairs). den gets the pair-sum once, PV runs per tile."""
                m, n, pa, pb = ent
                last = m == 7
                pad = paddp.tile([P, QR], bf16, tag="pa", name=f"pad{m}")
                nc.gpsimd.tensor_tensor(pad[:, 0:n], pa, pb, op=OP.add)
                nc.tensor.matmul(
                    den[:, 0:n], ones128, pad[:, 0:n],
                    start=(m == 0), stop=last, skip_group_check=True,
                )
                nc.tensor.matmul(
                    yt[:, 0:n], vA[:, 2 * m, kv, :], pa,
                    start=(m == 0), stop=False, skip_group_check=True,
                )
                nc.tensor.matmul(
                    yt[:, 0:n], vA[:, 2 * m + 1, kv, :], pb,
                    start=False, stop=last, skip_group_check=True,
                )

            for h in range(NH):
                kv = h // (NH // NKV)
                yt = psY.tile([P, QR], f32, tag="Y", name=f"yt{h}")
                den = psD.tile([P, QR], f32, tag="D", name=f"den{h}")
                dq = []
                for m in range(8):
                    n = 64 * (8 - m)
                    i = 7 - m  # masked slot for this kt pair
                    if n <= 256:
                        # packed: both kt of the pair in one psum bank
                        S = psS.tile([P, QR], f32, tag="S", name=f"S{h}_{m}")
                        for t in range(2):
                            nc.tensor.matmul(
                                S[:, t * n:(t + 1) * n],
                                kT[:, kv, (2 * m + t) * P:(2 * m + t + 1) * P],
                                qT[:, h, 0:n],
                                start=True, stop=True, skip_group_check=True,
                            )
                        if m == 4 and tail:
                            emit_tail()
                        pt = ptpool.tile([P, QR], bf16, tag="pt", name=f"pt{h}_{m}")
                        nc.scalar.activation(pt[:, 0:2 * n], S[:, 0:2 * n], AF.Exp)
                        nc.vector.tensor_tensor(
                            pt[:, n - 64:n], pt[:, n - 64:n], mk[:, i, 0, :],
                            op=OP.mult,
                        )
                        nc.vector.tensor_tensor(
                            pt[:, 2 * n - 64:2 * n], pt[:, 2 * n - 64:2 * n],
                            mk[:, i, 1, :], op=OP.mult,
                        )
                        ent = (m, n, pt[:, 0:n], pt[:, n:2 * n])
                    else:
                        pts = []
                        for t in range(2):
                            S = psS.tile([P, QR], f32, tag="S", name=f"S{h}_{m}_{t}")
                            nc.tensor.matmul(
                                S[:, 0:n],
                                kT[:, kv, (2 * m + t) * P:(2 * m + t + 1) * P],
                                qT[:, h, 0:n],
                                start=True, stop=True, skip_group_check=True,
                            )
                            if m == 0 and t == 0 and tail:
                                emit_tail()
                            pt = ptpool.tile(
                                [P, QR], bf16, tag="pt", name=f"pt{h}_{m}_{t}"
                            )
                            nc.scalar.activation(pt[:, 0:n], S[:, 0:n], AF.Exp)
                            nc.vector.tensor_tensor(
                                pt[:, n - 64:n], pt[:, n - 64:n], mk[:, i, t, :],
                                op=OP.mult,
                            )
                            pts.append(pt)
                        ent = (m, n, pts[0][:, 0:n], pts[1][:, 0:n])
                    dq.append(ent)
                    if len(dq) > 1:
                        emit_pair(dq.pop(0), yt, den, kv)
                while dq:
                    emit_pair(dq.pop(0), yt, den, kv)
                tail.append((yt, den, h))
            emit_tail()

            # ---------------- phase 4: output projection ----------------
            for s in range(4):
                w3 = w3s.pop(s)
                if s + 2 < 4:
                    w3s[s + 2] = load_wo_slab(s + 2)
                for rt in range(4):
                    ps = psS.tile([P, QR], f32, tag="S", name=f"o{s}_{rt}")
                    for h in range(NH):
                        nc.tensor.matmul(
                            ps, yT[:, h, rt * P:(rt + 1) * P], w3[:, h, :],
                            start=(h == 0), stop=(h == NH - 1),
                        )
                    ot = outpool.tile([P, QR], f32, tag="ot", name=f"ot{s}_{rt}")
                    nc.vector.tensor_copy(ot, ps)
                    nc.gpsimd.dma_start(
                        out=yo[rt * P:(rt + 1) * P, s * QR:(s + 1) * QR], in_=ot
                    )

    nc.compile()
    return nc


def _get_nc():
    if "nc" not in _CACHE:
        _CACHE["nc"] = _build()
    return _CACHE["nc"]


def _prep_in_maps(x, cos, sin, wq, wk, wv, wo):
    x = np.asarray(x, np.float32)
    cosr = np.asarray(cos, np.float32).reshape(T, HD // 2)
    sinr = np.asarray(sin, np.float32).reshape(T, HD // 2)
    wqb = np.ascontiguousarray(np.asarray(wq, np.float32).astype(BF16))
    wob = np.ascontiguousarray(np.asarray(wo, np.float32).astype(BF16))
    wkf = np.asarray(wk, np.float32)
    wvf = np.asarray(wv, np.float32)

    csf = np.ascontiguousarray(np.concatenate([cosr.T, sinr.T], axis=0))
    snf = np.ascontiguousarray(np.concatenate([sinr.T, cosr.T], axis=0))
    rpA_np, rpB_np = _rope_mats()

    maps = []
    for core in range(8):
        b, g = core // 4, core % 4
        qr = _qrows(g)
        xT_b = x[b].T
        maps.append({
            "xoT": np.ascontiguousarray(xT_b[:, qr].astype(BF16)),
            "xfT": np.ascontiguousarray(xT_b.astype(BF16)),
            "cso": np.ascontiguousarray(csf[:, qr]),
            "sno": np.ascontiguousarray(snf[:, qr]),
            "csf": csf,
            "snf": snf,
            "wq": wqb,
            "wks": np.ascontiguousarray(wkf[:, g * HD:(g + 1) * HD].astype(BF16)),
            "wvs": np.ascontiguousarray(wvf[:, g * HD:(g + 1) * HD].astype(BF16)),
            "wo": wob,
            "msk": _mask01(g),
            "rpA": rpA_np,
            "rpB": rpB_np,
        })
    return maps


def kernel(x, cos, sin, wq, wk, wv, wo):
    from concourse.bass_utils import run_bass_kernel_spmd

    nc = _get_nc()
    maps = _prep_in_maps(x, cos, sin, wq, wk, wv, wo)
    _CACHE["in_maps"] = maps
    res = run_bass_kernel_spmd(nc, maps, list(range(8)))
    y = np.empty((B, T, C), np.float32)
    for core in range(8):
        b, g = core // 4, core % 4
        y[b][_qrows(g)] = res.results[core]["yo"]
    return y


# revision 12
# speedup vs baseline: 1.2926x; 1.0685x over previous
"""Causal self-attention (GQA, rope, qk-rmsnorm) on 8 TRN2 NeuronCores.

Sharding: core = (b, g), b = core // 4 (batch), g = core % 4.
Each core owns 8 interleaved 64-row query slots of its batch (balanced
causal assignment), computes Q for those 512 rows (all 16 heads), K/V
for kv-head g only (sharded 4-way), all-gathers K/V within its 4-core
batch group (overlapped with the Q projection), runs attention for all
heads over its own queries, and projects its 512-row output slice
locally (no output collective).

All PE-facing tensors stay transposed ([feature, token]); rope's
cross-partition half-swap runs on the PE via two constant combine
matrices, rms-norm partition sums via a ones matmul, and the causal
mask is a post-exp 0/1 multiply on the vector engine. The host
pre-transposes x and converts weights to bf16.

Engine queues: sync = pure loads, scalar = activation compute (+ the
v DMA-transposes), gpsimd = dependent stores, the collective, gather-
ins, and the den pair-adds.
"""

import sys

if "/opt/trn_rl_repo" not in sys.path:
    sys.path.insert(0, "/opt/trn_rl_repo")

import numpy as np
import ml_dtypes

BF16 = ml_dtypes.bfloat16

B, T, C = 2, 2048, 2048
NH, NKV = 16, 4
HD = 128
P = 128
NCT = C // P           # 16 contraction tiles
QR = 512               # own query rows per core
NKT = T // P           # 16 key tiles
A_SHAPE = [16, 14, 12, 10, 8, 6, 4, 2]  # uniform key-tile count per slot
EPS = float(np.finfo(np.float32).eps)
USE_DSQRT = False      # Dsqrt has no ACT table on this stack; use Sqrt+recip

_CACHE = {}


def _slots64(g):
    """Own 64-row query chunks, descending. Balanced: sum of causal
    key-needs is equal across g."""
    return [31 - g, 24 + g, 23 - g, 16 + g, 15 - g, 8 + g, 7 - g, g]


def _qrows(g):
    return np.concatenate([np.arange(c * 64, (c + 1) * 64) for c in _slots64(g)])


def _mask01(g):
    """0/1 mask, applied to exp'd score tiles: for slot i the program
    masks key tiles A_SHAPE[i]-2 and A_SHAPE[i]-1 (j = 0, 1); entry is
    1 where key <= query."""
    m = np.zeros((8, 2, P, 64), np.float32)
    sl = _slots64(g)
    for i in range(8):
        q = sl[i] * 64 + np.arange(64)[None, :]
        for j in range(2):
            kt = A_SHAPE[i] - 2 + j
            k = kt * P + np.arange(P)[:, None]
            m[i, j] = (k <= q).astype(np.float32)
    return np.ascontiguousarray(m.astype(BF16))


def _rope_mats():
    """ro = A @ m1 + B @ m2 with m1 = q*[cos;sin], m2 = q*[sin;cos]:
    ro[0:64] = m1[0:64] + m1[64:128]; ro[64:128] = m2[64:128] - m2[0:64].
    Returned transposed ([contraction, out_partition]) for use as lhsT."""
    Am = np.zeros((P, P), np.float32)
    Bm = np.zeros((P, P), np.float32)
    for o in range(64):
        Am[o, o] = 1.0
        Am[o + 64, o] = 1.0
    for o in range(64, 128):
        Bm[o, o] = 1.0
        Bm[o - 64, o] = -1.0
    return np.ascontiguousarray(Am.astype(BF16)), np.ascontiguousarray(Bm.astype(BF16))


def _build():
    import concourse.bacc as bacc
    import concourse.mybir as mybir
    import concourse.tile as tile

    f32 = mybir.dt.float32
    bf16 = mybir.dt.bfloat16
    AF = mybir.ActivationFunctionType
    OP = mybir.AluOpType

    nc = bacc.Bacc("TRN2", target_bir_lowering=False, debug=False, num_devices=8)

    xoT = nc.dram_tensor("xoT", [C, QR], bf16, kind="ExternalInput").ap()
    xfT = nc.dram_tensor("xfT", [C, T], bf16, kind="ExternalInput").ap()
    cso = nc.dram_tensor("cso", [P, QR], f32, kind="ExternalInput").ap()
    sno = nc.dram_tensor("sno", [P, QR], f32, kind="ExternalInput").ap()
    csf = nc.dram_tensor("csf", [P, T], f32, kind="ExternalInput").ap()
    snf = nc.dram_tensor("snf", [P, T], f32, kind="ExternalInput").ap()
    wq = nc.dram_tensor("wq", [C, C], bf16, kind="ExternalInput").ap()
    wks = nc.dram_tensor("wks", [C, HD], bf16, kind="ExternalInput").ap()
    wvs = nc.dram_tensor("wvs", [C, HD], bf16, kind="ExternalInput").ap()
    wo = nc.dram_tensor("wo", [C, C], bf16, kind="ExternalInput").ap()
    msk = nc.dram_tensor("msk", [8, 2, P, 64], bf16, kind="ExternalInput").ap()
    rpA = nc.dram_tensor("rpA", [P, P], bf16, kind="ExternalInput").ap()
    rpB = nc.dram_tensor("rpB", [P, P], bf16, kind="ExternalInput").ap()
    yo = nc.dram_tensor("yo", [QR, C], f32, kind="ExternalOutput").ap()

    with tile.TileContext(nc) as tc:
        with (
            tc.tile_pool(name="singles", bufs=1) as singles,
            tc.tile_pool(name="big", bufs=1) as bigpool,
            tc.tile_pool(name="xc", bufs=2) as xcpool,
            tc.tile_pool(name="wqh", bufs=4) as wqpool,
            tc.tile_pool(name="rope", bufs=3) as ropep,
            tc.tile_pool(name="stage", bufs=2) as stagep,
            tc.tile_pool(name="pt", bufs=6) as ptpool,
            tc.tile_pool(name="padd", bufs=3) as paddp,
            tc.tile_pool(name="small", bufs=2) as smallp,
            tc.tile_pool(name="outs", bufs=2) as outpool,
            tc.tile_pool(name="psS", bufs=4, space="PSUM") as psS,
            tc.tile_pool(name="psY", bufs=2, space="PSUM") as psY,
            tc.tile_pool(name="psD", bufs=2, space="PSUM") as psD,
            tc.tile_pool(name="dram", bufs=1, space="DRAM") as drampool,
        ):
            ones128 = singles.tile([P, P], bf16)
            nc.vector.memset(ones128, 1.0)
            eps_q = singles.tile([P, 1], f32)
            nc.vector.memset(eps_q, HD * EPS / 4.0 if USE_DSQRT else HD * EPS)
            eps_k = singles.tile([P, 1], f32)
            nc.vector.memset(eps_k, EPS / 4.0 if USE_DSQRT else EPS)
            sc_q = 0.25 if USE_DSQRT else 1.0
            sc_k = (1.0 / (4.0 * HD)) if USE_DSQRT else 1.0 / HD
            # load order matters: sync is FIFO — K/V weights and the first
            # x chunk first so the PE starts ASAP; cos/sin split per chunk
            wks_sb = singles.tile([P, NCT, HD], bf16)
            nc.sync.dma_start(out=wks_sb, in_=wks.rearrange("(a p) n -> p a n", p=P))
            wvs_sb = singles.tile([P, NCT, HD], bf16)
            nc.sync.dma_start(out=wvs_sb, in_=wvs.rearrange("(a p) n -> p a n", p=P))
            rpA_sb = singles.tile([P, P], bf16)
            rpB_sb = singles.tile([P, P], bf16)
            csf_sb = singles.tile([P, T], f32)
            snf_sb = singles.tile([P, T], f32)
            cso_sb = singles.tile([P, QR], f32)
            sno_sb = singles.tile([P, QR], f32)
            mk = singles.tile([P, 8, 2, 64], bf16)

            kv_in = drampool.tile([P, 2 * T], bf16, tag="kvin")
            kv_out = drampool.tile([NKV * P, 2 * T], bf16, tag="kvout")

            def rope_rms(ps, cs_ap, sn_ap, out_ap, scale, bias_ap, nm):
                """ps: [128, 512] psum f32 = projected [head_dim, tok].
                Applies rope (PE combine) + rms-norm, writes bf16 out_ap."""
                m1 = ropep.tile([P, QR], bf16, tag="m1", name=f"m1{nm}")
                m2 = ropep.tile([P, QR], bf16, tag="m2", name=f"m2{nm}")
                nc.vector.tensor_tensor(m1, ps, cs_ap, op=OP.mult)
                nc.vector.tensor_tensor(m2, ps, sn_ap, op=OP.mult)
                ro = psY.tile([P, QR], f32, tag="Y", name=f"ro{nm}")
                nc.tensor.matmul(ro, rpA_sb, m1, start=True, stop=False)
                nc.tensor.matmul(ro, rpB_sb, m2, start=False, stop=True)
                sq = ropep.tile([P, QR], bf16, tag="sq", name=f"sq{nm}")
                nc.scalar.activation(sq, ro, AF.Square)
                ss = psD.tile([P, QR], f32, tag="D", name=f"ss{nm}")
                nc.tensor.matmul(ss, ones128, sq, start=True, stop=True)
                rinv = ropep.tile([P, QR], f32, tag="rinv", name=f"rinv{nm}")
                if USE_DSQRT:
                    nc.scalar.activation(rinv, ss, AF.Dsqrt, bias=bias_ap, scale=scale)
                else:
                    rms = ropep.tile([P, QR], f32, tag="rms", name=f"rms{nm}")
                    nc.scalar.activation(rms, ss, AF.Sqrt, bias=bias_ap, scale=scale)
                    nc.vector.reciprocal_approx_fast(rinv, rms)
                nc.vector.tensor_tensor(out_ap, ro, rinv, op=OP.mult)

            # ---------------- phase 1: K/V projection (own kv head) -----
            for j in range(4):
                xc = xcpool.tile([P, NCT, QR], bf16, tag="xc", name=f"xc{j}")
                for hf in range(2):
                    nc.sync.dma_start(
                        out=xc[:, 8 * hf:8 * hf + 8, :],
                        in_=xfT[:, j * QR:(j + 1) * QR].rearrange(
                            "(a p) n -> p a n", p=P
                        )[:, 8 * hf:8 * hf + 8, :],
                    )
                if j == 0:
                    nc.sync.dma_start(out=rpA_sb, in_=rpA)
                    nc.sync.dma_start(out=rpB_sb, in_=rpB)
                nc.sync.dma_start(
                    out=csf_sb[:, j * QR:(j + 1) * QR],
                    in_=csf[:, j * QR:(j + 1) * QR],
                )
                nc.sync.dma_start(
                    out=snf_sb[:, j * QR:(j + 1) * QR],
                    in_=snf[:, j * QR:(j + 1) * QR],
                )
                kp = psS.tile([P, QR], f32, tag="S", name=f"kp{j}")
                for ct in range(NCT):
                    nc.tensor.matmul(
                        kp, wks_sb[:, ct, :], xc[:, ct, :],
                        start=(ct == 0), stop=(ct == NCT - 1),
                    )
                kst = stagep.tile([P, QR], bf16, tag="kst", name=f"kst{j}")
                rope_rms(
                    kp, csf_sb[:, j * QR:(j + 1) * QR],
                    snf_sb[:, j * QR:(j + 1) * QR], kst, sc_k, eps_k, f"k{j}",
                )
                nc.gpsimd.dma_start(out=kv_in[:, j * QR:(j + 1) * QR], in_=kst)

                # V directly in [tok, d] layout (lhsT = x tile): no transpose
                vp = psS.tile([P, 4, P], f32, tag="S", name=f"vp{j}")
                for tk in range(4):
                    for ct in range(NCT):
                        nc.tensor.matmul(
                            vp[:, tk, :],
                            xc[:, ct, tk * P:(tk + 1) * P],
                            wvs_sb[:, ct, :],
                            start=(ct == 0), stop=(ct == NCT - 1),
                            skip_group_check=True,
                        )
                vtmp = stagep.tile([P, 4, P], bf16, tag="vtmp", name=f"vtmp{j}")
                nc.vector.tensor_copy(vtmp, vp)
                nc.gpsimd.dma_start(
                    out=kv_in[:, T + j * QR:T + (j + 1) * QR], in_=vtmp
                )

            nc.gpsimd.collective_compute(
                "AllGather",
                OP.bypass,
                replica_groups=[[0, 1, 2, 3], [4, 5, 6, 7]],
                ins=[kv_in.opt()],
                outs=[kv_out.opt()],
            )

            # K/V gathered from the group (gpsimd: right after the trigger)
            kT = bigpool.tile([P, NKV, T], bf16, tag="kT")
            vA = bigpool.tile([P, NKT, NKV, HD], bf16, tag="vA")
            for kv in range(NKV):
                nc.gpsimd.dma_start(
                    out=kT[:, kv, :], in_=kv_out[kv * P:(kv + 1) * P, 0:T]
                )
                nc.gpsimd.dma_start(
                    out=vA[:, :, kv, :],
                    in_=kv_out[kv * P:(kv + 1) * P, T:2 * T].rearrange(
                        "p (t d) -> p t d", d=HD
                    ),
                )

            # ---------------- phase 2: Q projection (all heads) ---------
            nc.sync.dma_start(out=cso_sb, in_=cso)
            nc.sync.dma_start(out=sno_sb, in_=sno)
            nc.sync.dma_start(out=mk, in_=msk.rearrange("i j p n -> p i j n"))
            # shares the xc pool (xc chunks are dead by now)
            xo_sb = xcpool.tile([P, NCT, QR], bf16, tag="xc", name="xo")
            for hf in range(4):
                nc.sync.dma_start(
                    out=xo_sb[:, 4 * hf:4 * hf + 4, :],
                    in_=xoT.rearrange("(a p) n -> p a n", p=P)[:, 4 * hf:4 * hf + 4, :],
                )
            qT = bigpool.tile([P, NH, QR], bf16, tag="qT")
            for h in range(NH):
                wqh = wqpool.tile([P, NCT, HD], bf16, tag="wqh", name=f"wqh{h}")
                nc.sync.dma_start(
                    out=wqh,
                    in_=wq[:, h * HD:(h + 1) * HD].rearrange("(a p) n -> p a n", p=P),
                )
                qp = psS.tile([P, QR], f32, tag="S", name=f"qp{h}")
                for ct in range(NCT):
                    nc.tensor.matmul(
                        qp, wqh[:, ct, :], xo_sb[:, ct, :],
                        start=(ct == 0), stop=(ct == NCT - 1),
                    )
                rope_rms(qp, cso_sb, sno_sb, qT[:, h, :], sc_q, eps_q, f"q{h}")

            # ---------------- phase 3: attention ------------------------
            yT = bigpool.tile([P, NH, QR], bf16, tag="yT")

            def load_wo_slab(s):
                # shares the xc pool; slab s=1 evicts xo_sb (read-dep safe)
                w3 = xcpool.tile([P, NCT, QR], bf16, tag="xc", name=f"wo{s}")
                for gr in range(4):
                    nc.sync.dma_start(
                        out=w3[:, 4 * gr:4 * gr + 4, :],
                        in_=wo[:, s * QR:(s + 1) * QR].rearrange(
                            "(a p) n -> p a n", p=P
                        )[:, 4 * gr:4 * gr + 4, :],
                    )
                return w3

            w3s = {0: load_wo_slab(0), 1: load_wo_slab(1)}

            tail = []

            def emit_tail():
                yt, den, h = tail.pop(0)
                rinv = smallp.tile([P, QR], f32, tag="rq", name=f"rq{h}")
                nc.vector.reciprocal_approx_fast(rinv, den)
                nc.vector.tensor_tensor(yT[:, h, :], yt, rinv, op=OP.mult)

            def emit_pair(ent, yt, den, kv):
                """ent = (m, n, pa, pb) with pa/pb the exp'd tiles for
                kt = 2m, 2m+1 (pb is a column-slice view for packed
                pairs). den gets the pair-sum once, PV runs per tile."""
                m, n, pa, pb = ent
                last = m == 7
                pad = paddp.tile([P, QR], bf16, tag="pa", name=f"pad{m}")
                nc.gpsimd.tensor_tensor(pad[:, 0:n], pa, pb, op=OP.add)
                nc.tensor.matmul(
                    den[:, 0:n], ones128, pad[:, 0:n],
                    start=(m == 0), stop=last, skip_group_check=True,
                )
                nc.tensor.matmul(
                    yt[:, 0:n], vA[:, 2 * m, kv, :], pa,
                    start=(m == 0), stop=False, skip_group_check=True,
                )
                nc.tensor.matmul(
                    yt[:, 0:n], vA[:, 2 * m + 1, kv, :], pb,
                    start=False, stop=last, skip_group_check=True,
                )

            for h in range(NH):
                kv = h // (NH // NKV)
                yt = psY.tile([P, QR], f32, tag="Y", name=f"yt{h}")
                den = psD.tile([P, QR], f32, tag="D", name=f"den{h}")
                dq = []
                for m in range(8):
                    n = 64 * (8 - m)
                    i = 7 - m  # masked slot for this kt pair
                    if n <= 256:
                        # packed: both kt of the pair in one psum bank
                        S = psS.tile([P, QR], f32, tag="S", name=f"S{h}_{m}")
                        for t in range(2):
                            nc.tensor.matmul(
                                S[:, t * n:(t + 1) * n],
                                kT[:, kv, (2 * m + t) * P:(2 * m + t + 1) * P],
                                qT[:, h, 0:n],
                                start=True, stop=True, skip_group_check=True,
                            )
                        if m == 4 and tail:
                            emit_tail()
                        pt = ptpool.tile([P, QR], bf16, tag="pt", name=f"pt{h}_{m}")
                        nc.scalar.activation(pt[:, 0:2 * n], S[:, 0:2 * n], AF.Exp)
                        nc.vector.tensor_tensor(
                            pt[:, n - 64:n], pt[:, n - 64:n], mk[:, i, 0, :],
                            op=OP.mult,
                        )
                        nc.vector.tensor_tensor(
                            pt[:, 2 * n - 64:2 * n], pt[:, 2 * n - 64:2 * n],
                            mk[:, i, 1, :], op=OP.mult,
                        )
                        ent = (m, n, pt[:, 0:n], pt[:, n:2 * n])
                    else:
                        pts = []
                        for t in range(2):
                            S = psS.tile([P, QR], f32, tag="S", name=f"S{h}_{m}_{t}")
                            nc.tensor.matmul(
                                S[:, 0:n],
                                kT[:, kv, (2 * m + t) * P:(2 * m + t + 1) * P],
                                qT[:, h, 0:n],
                                start=True, stop=True, skip_group_check=True,
                            )
                            if m == 0 and t == 0 and tail:
                                emit_tail()
                            pt = ptpool.tile(
                                [P, QR], bf16, tag="pt", name=f"pt{h}_{m}_{t}"
                            )
                            nc.scalar.activation(pt[:, 0:n], S[:, 0:n], AF.Exp)
                            nc.vector.tensor_tensor(
                                pt[:, n - 64:n], pt[:, n - 64:n], mk[:, i, t, :],
                                op=OP.mult,
                            )
                            pts.append(pt)
                        ent = (m, n, pts[0][:, 0:n], pts[1][:, 0:n])
                    dq.append(ent)
                    if len(dq) > 1:
                        emit_pair(dq.pop(0), yt, den, kv)
                while dq:
                    emit_pair(dq.pop(0), yt, den, kv)
                tail.append((yt, den, h))
            emit_tail()

            # ---------------- phase 4: output projection ----------------
            for s in range(4):
                w3 = w3s.pop(s)
                if s + 2 < 4:
                    w3s[s + 2] = load_wo_slab(s + 2)
                for rt in range(4):
                    ps = psS.tile([P, QR], f32, tag="S", name=f"o{s}_{rt}")
                    for h in range(NH):
                        nc.tensor.matmul(
                            ps, yT[:, h, rt * P:(rt + 1) * P], w3[:, h, :],
                            start=(h == 0), stop=(h == NH - 1),
                        )
                    ot = outpool.tile([P, QR], f32, tag="ot", name=f"ot{s}_{rt}")
                    nc.vector.tensor_copy(ot, ps)
                    nc.gpsimd.dma_start(
                        out=yo[rt * P:(rt + 1) * P, s * QR:(s + 1) * QR], in_=ot
                    )

    nc.compile()
    return nc


def _get_nc():
    if "nc" not in _CACHE:
        _CACHE["nc"] = _build()
    return _CACHE["nc"]


def _prep_in_maps(x, cos, sin, wq, wk, wv, wo):
    x = np.asarray(x, np.float32)
    cosr = np.asarray(cos, np.float32).reshape(T, HD // 2)
    sinr = np.asarray(sin, np.float32).reshape(T, HD // 2)
    wqb = np.ascontiguousarray(np.asarray(wq, np.float32).astype(BF16))
    wob = np.ascontiguousarray(np.asarray(wo, np.float32).astype(BF16))
    wkf = np.asarray(wk, np.float32)
    wvf = np.asarray(wv, np.float32)

    csf = np.ascontiguousarray(np.concatenate([cosr.T, sinr.T], axis=0))
    snf = np.ascontiguousarray(np.concatenate([sinr.T, cosr.T], axis=0))
    rpA_np, rpB_np = _rope_mats()

    maps = []
    for core in range(8):
        b, g = core // 4, core % 4
        qr = _qrows(g)
        xT_b = x[b].T
        maps.append({
            "xoT": np.ascontiguousarray(xT_b[:, qr].astype(BF16)),
            "xfT": np.ascontiguousarray(xT_b.astype(BF16)),
            "cso": np.ascontiguousarray(csf[:, qr]),
            "sno": np.ascontiguousarray(snf[:, qr]),
            "csf": csf,
            "snf": snf,
            "wq": wqb,
            "wks": np.ascontiguousarray(wkf[:, g * HD:(g + 1) * HD].astype(BF16)),
            "wvs": np.ascontiguousarray(wvf[:, g * HD:(g + 1) * HD].astype(BF16)),
            "wo": wob,
            "msk": _mask01(g),
            "rpA": rpA_np,
            "rpB": rpB_np,
        })
    return maps


def kernel(x, cos, sin, wq, wk, wv, wo):
    from concourse.bass_utils import run_bass_kernel_spmd

    nc = _get_nc()
    maps = _prep_in_maps(x, cos, sin, wq, wk, wv, wo)
    _CACHE["in_maps"] = maps
    res = run_bass_kernel_spmd(nc, maps, list(range(8)))
    y = np.empty((B, T, C), np.float32)
    for core in range(8):
        b, g = core // 4, core % 4
        y[b][_qrows(g)] = res.results[core]["yo"]
    return y


# revision 17
# speedup vs baseline: 1.3156x; 1.0177x over previous
"""Causal self-attention (GQA, rope, qk-rmsnorm) on 8 TRN2 NeuronCores.

Sharding: core = (b, g), b = core // 4 (batch), g = core % 4.
Each core owns 8 interleaved 64-row query slots of its batch (balanced
causal assignment), computes Q for those 512 rows (all 16 heads), K/V
for kv-head g only (sharded 4-way), all-gathers K/V within its 4-core
batch group (overlapped with the Q projection), runs attention for all
heads over its own queries, and projects its 512-row output slice
locally (no output collective).

All PE-facing tensors stay transposed ([feature, token]); rope's
cross-partition half-swap runs on the PE via two constant combine
matrices, rms-norm partition sums via a ones matmul, and the causal
mask is a post-exp 0/1 multiply on the vector engine. The host
pre-transposes x and converts weights to bf16.

Engine queues: sync = pure loads, scalar = activation compute (+ the
v DMA-transposes), gpsimd = dependent stores, the collective, gather-
ins, and the den pair-adds.
"""

import sys

if "/opt/trn_rl_repo" not in sys.path:
    sys.path.insert(0, "/opt/trn_rl_repo")

import numpy as np
import ml_dtypes

BF16 = ml_dtypes.bfloat16

B, T, C = 2, 2048, 2048
NH, NKV = 16, 4
HD = 128
P = 128
NCT = C // P           # 16 contraction tiles
QR = 512               # own query rows per core
NKT = T // P           # 16 key tiles
A_SHAPE = [16, 14, 12, 10, 8, 6, 4, 2]  # uniform key-tile count per slot
EPS = float(np.finfo(np.float32).eps)
USE_DSQRT = False      # Dsqrt has no ACT table on this stack; use Sqrt+recip

_CACHE = {}


def _slots64(g):
    """Own 64-row query chunks, descending. Balanced: sum of causal
    key-needs is equal across g."""
    return [31 - g, 24 + g, 23 - g, 16 + g, 15 - g, 8 + g, 7 - g, g]


def _qrows(g):
    return np.concatenate([np.arange(c * 64, (c + 1) * 64) for c in _slots64(g)])


def _mask01(g):
    """Additive causal mask, accumulated into the score psum on the PE:
    for slot i the program masks key tiles A_SHAPE[i]-2 and A_SHAPE[i]-1
    (j = 0, 1); entry is 0 where key <= query else -1e9."""
    m = np.zeros((8, 2, P, 64), np.float32)
    sl = _slots64(g)
    for i in range(8):
        q = sl[i] * 64 + np.arange(64)[None, :]
        for j in range(2):
            kt = A_SHAPE[i] - 2 + j
            k = kt * P + np.arange(P)[:, None]
            m[i, j] = np.where(k <= q, 0.0, -1.0e9)
    return np.ascontiguousarray(m.astype(BF16))


def _rope_mats():
    """ro = A @ m1 + B @ m2 with m1 = q*[cos;sin], m2 = q*[sin;cos]:
    ro[0:64] = m1[0:64] + m1[64:128]; ro[64:128] = m2[64:128] - m2[0:64].
    Returned transposed ([contraction, out_partition]) for use as lhsT."""
    Am = np.zeros((P, P), np.float32)
    Bm = np.zeros((P, P), np.float32)
    for o in range(64):
        Am[o, o] = 1.0
        Am[o + 64, o] = 1.0
    for o in range(64, 128):
        Bm[o, o] = 1.0
        Bm[o - 64, o] = -1.0
    return np.ascontiguousarray(Am.astype(BF16)), np.ascontiguousarray(Bm.astype(BF16))


def _build():
    import concourse.bacc as bacc
    import concourse.mybir as mybir
    import concourse.tile as tile
    from concourse.masks import make_identity

    f32 = mybir.dt.float32
    bf16 = mybir.dt.bfloat16
    AF = mybir.ActivationFunctionType
    OP = mybir.AluOpType

    nc = bacc.Bacc("TRN2", target_bir_lowering=False, debug=False, num_devices=8)

    xoT = nc.dram_tensor("xoT", [P, NCT, QR], bf16, kind="ExternalInput").ap()
    xfT = nc.dram_tensor("xfT", [4, P, NCT, QR], bf16, kind="ExternalInput").ap()
    cso = nc.dram_tensor("cso", [P, QR], f32, kind="ExternalInput").ap()
    sno = nc.dram_tensor("sno", [P, QR], f32, kind="ExternalInput").ap()
    csf = nc.dram_tensor("csf", [P, T], f32, kind="ExternalInput").ap()
    snf = nc.dram_tensor("snf", [P, T], f32, kind="ExternalInput").ap()
    wq = nc.dram_tensor("wq", [NH, P, NCT, HD], bf16, kind="ExternalInput").ap()
    wks = nc.dram_tensor("wks", [P, NCT, HD], bf16, kind="ExternalInput").ap()
    wvs = nc.dram_tensor("wvs", [P, NCT, HD], bf16, kind="ExternalInput").ap()
    wo = nc.dram_tensor("wo", [4, P, NCT, QR], bf16, kind="ExternalInput").ap()
    msk = nc.dram_tensor("msk", [P, 8, 2, 64], bf16, kind="ExternalInput").ap()
    rpA = nc.dram_tensor("rpA", [P, P], bf16, kind="ExternalInput").ap()
    rpB = nc.dram_tensor("rpB", [P, P], bf16, kind="ExternalInput").ap()
    yo = nc.dram_tensor("yo", [QR, C], f32, kind="ExternalOutput").ap()

    with tile.TileContext(nc) as tc:
        with (
            tc.tile_pool(name="singles", bufs=1) as singles,
            tc.tile_pool(name="big", bufs=1) as bigpool,
            tc.tile_pool(name="xc", bufs=2) as xcpool,
            tc.tile_pool(name="wqh", bufs=8) as wqpool,
            tc.tile_pool(name="rope", bufs=3) as ropep,
            tc.tile_pool(name="stage", bufs=2) as stagep,
            tc.tile_pool(name="pt", bufs=6) as ptpool,
            tc.tile_pool(name="padd", bufs=3) as paddp,
            tc.tile_pool(name="small", bufs=2) as smallp,
            tc.tile_pool(name="outs", bufs=2) as outpool,
            tc.tile_pool(name="psS", bufs=4, space="PSUM") as psS,
            tc.tile_pool(name="psY", bufs=2, space="PSUM") as psY,
            tc.tile_pool(name="psD", bufs=2, space="PSUM") as psD,
            tc.tile_pool(name="dram", bufs=1, space="DRAM") as drampool,
        ):
            ones128 = singles.tile([P, P], bf16)
            nc.vector.memset(ones128, 1.0)
            ident = singles.tile([P, P], bf16)
            make_identity(nc, ident)
            eps_q = singles.tile([P, 1], f32)
            nc.vector.memset(eps_q, HD * EPS / 4.0 if USE_DSQRT else HD * EPS)
            eps_k = singles.tile([P, 1], f32)
            nc.vector.memset(eps_k, EPS / 4.0 if USE_DSQRT else EPS)
            sc_q = 0.25 if USE_DSQRT else 1.0
            sc_k = (1.0 / (4.0 * HD)) if USE_DSQRT else 1.0 / HD
            # load order matters: sync is FIFO — K/V weights and the first
            # x chunk first so the PE starts ASAP; cos/sin split per chunk
            wks_sb = singles.tile([P, NCT, HD], bf16)
            nc.sync.dma_start(out=wks_sb, in_=wks)
            wvs_sb = singles.tile([P, NCT, HD], bf16)
            nc.sync.dma_start(out=wvs_sb, in_=wvs)
            rpA_sb = singles.tile([P, P], bf16)
            rpB_sb = singles.tile([P, P], bf16)
            csf_sb = singles.tile([P, T], f32)
            snf_sb = singles.tile([P, T], f32)
            cso_sb = singles.tile([P, QR], f32)
            sno_sb = singles.tile([P, QR], f32)
            mk = singles.tile([P, 8, 2, 64], bf16)

            kv_in = drampool.tile([P, 2 * T], bf16, tag="kvin")
            kv_out = drampool.tile([NKV * P, 2 * T], bf16, tag="kvout")

            def rope_rms(ps, cs_ap, sn_ap, out_ap, scale, bias_ap, nm):
                """ps: [128, 512] psum f32 = projected [head_dim, tok].
                Applies rope (PE combine) + rms-norm, writes bf16 out_ap."""
                m1 = ropep.tile([P, QR], bf16, tag="m1", name=f"m1{nm}")
                m2 = ropep.tile([P, QR], bf16, tag="m2", name=f"m2{nm}")
                nc.vector.tensor_tensor(m1, ps, cs_ap, op=OP.mult)
                nc.vector.tensor_tensor(m2, ps, sn_ap, op=OP.mult)
                ro = psY.tile([P, QR], f32, tag="Y", name=f"ro{nm}")
                nc.tensor.matmul(ro, rpA_sb, m1, start=True, stop=False)
                nc.tensor.matmul(ro, rpB_sb, m2, start=False, stop=True)
                sq = ropep.tile([P, QR], bf16, tag="sq", name=f"sq{nm}")
                nc.scalar.activation(sq, ro, AF.Square)
                ss = psD.tile([P, QR], f32, tag="D", name=f"ss{nm}")
                nc.tensor.matmul(ss, ones128, sq, start=True, stop=True)
                rinv = ropep.tile([P, QR], f32, tag="rinv", name=f"rinv{nm}")
                if USE_DSQRT:
                    nc.scalar.activation(rinv, ss, AF.Dsqrt, bias=bias_ap, scale=scale)
                else:
                    rms = ropep.tile([P, QR], f32, tag="rms", name=f"rms{nm}")
                    nc.scalar.activation(rms, ss, AF.Sqrt, bias=bias_ap, scale=scale)
                    nc.vector.reciprocal_approx_fast(rinv, rms)
                nc.vector.tensor_tensor(out_ap, ro, rinv, op=OP.mult)

            # ---------------- phase 1: K/V projection (own kv head) -----
            for j in range(4):
                xc = xcpool.tile([P, NCT, QR], bf16, tag="xc", name=f"xc{j}")
                for hf in range(2):
                    nc.sync.dma_start(
                        out=xc[:, 8 * hf:8 * hf + 8, :],
                        in_=xfT[j][:, 8 * hf:8 * hf + 8, :],
                    )
                if j == 0:
                    nc.sync.dma_start(out=rpA_sb, in_=rpA)
                    nc.sync.dma_start(out=rpB_sb, in_=rpB)
                nc.sync.dma_start(
                    out=csf_sb[:, j * QR:(j + 1) * QR],
                    in_=csf[:, j * QR:(j + 1) * QR],
                )
                nc.sync.dma_start(
                    out=snf_sb[:, j * QR:(j + 1) * QR],
                    in_=snf[:, j * QR:(j + 1) * QR],
                )
                kp = psS.tile([P, QR], f32, tag="S", name=f"kp{j}")
                for ct in range(NCT):
                    nc.tensor.matmul(
                        kp, wks_sb[:, ct, :], xc[:, ct, :],
                        start=(ct == 0), stop=(ct == NCT - 1),
                    )
                kst = stagep.tile([P, QR], bf16, tag="kst", name=f"kst{j}")
                rope_rms(
                    kp, csf_sb[:, j * QR:(j + 1) * QR],
                    snf_sb[:, j * QR:(j + 1) * QR], kst, sc_k, eps_k, f"k{j}",
                )
                nc.gpsimd.dma_start(out=kv_in[:, j * QR:(j + 1) * QR], in_=kst)

                vp = psS.tile([P, QR], f32, tag="S", name=f"vp{j}")
                for ct in range(NCT):
                    nc.tensor.matmul(
                        vp, wvs_sb[:, ct, :], xc[:, ct, :],
                        start=(ct == 0), stop=(ct == NCT - 1),
                    )
                vtmp = stagep.tile([P, QR], bf16, tag="vtmp", name=f"vtmp{j}")
                nc.vector.tensor_copy(vtmp, vp)
                vstage = stagep.tile([P, 4, P], bf16, tag="vst", name=f"vst{j}")
                for tk in range(4):
                    nc.scalar.dma_start_transpose(
                        vstage[:, tk, :], vtmp[:, tk * P:(tk + 1) * P]
                    )
                nc.gpsimd.dma_start(
                    out=kv_in[:, T + j * QR:T + (j + 1) * QR], in_=vstage
                )

            nc.gpsimd.collective_compute(
                "AllGather",
                OP.bypass,
                replica_groups=[[0, 1, 2, 3], [4, 5, 6, 7]],
                ins=[kv_in.opt()],
                outs=[kv_out.opt()],
            )

            # K/V gathered from the group (gpsimd: right after the trigger)
            kT = bigpool.tile([P, NKV, T], bf16, tag="kT")
            vA = bigpool.tile([P, NKV, NKT, HD], bf16, tag="vA")
            for kv in range(NKV):
                nc.gpsimd.dma_start(
                    out=kT[:, kv, :], in_=kv_out[kv * P:(kv + 1) * P, 0:T]
                )
                nc.gpsimd.dma_start(
                    out=vA[:, kv, :, :],
                    in_=kv_out[kv * P:(kv + 1) * P, T:2 * T].rearrange(
                        "p (t d) -> p t d", d=HD
                    ),
                )

            # ---------------- phase 2: Q projection (all heads) ---------
            nc.sync.dma_start(out=cso_sb, in_=cso)
            nc.sync.dma_start(out=sno_sb, in_=sno)
            nc.sync.dma_start(out=mk, in_=msk)
            # shares the xc pool (xc chunks are dead by now)
            xo_sb = xcpool.tile([P, NCT, QR], bf16, tag="xc", name="xo")
            for hf in range(4):
                nc.sync.dma_start(
                    out=xo_sb[:, 4 * hf:4 * hf + 4, :],
                    in_=xoT[:, 4 * hf:4 * hf + 4, :],
                )
            qT = bigpool.tile([P, NH, QR], bf16, tag="qT")
            for h in range(NH):
                wqh = wqpool.tile([P, NCT, HD], bf16, tag="wqh", name=f"wqh{h}")
                nc.sync.dma_start(out=wqh, in_=wq[h])
                qp = psS.tile([P, QR], f32, tag="S", name=f"qp{h}")
                for ct in range(NCT):
                    nc.tensor.matmul(
                        qp, wqh[:, ct, :], xo_sb[:, ct, :],
                        start=(ct == 0), stop=(ct == NCT - 1),
                    )
                rope_rms(qp, cso_sb, sno_sb, qT[:, h, :], sc_q, eps_q, f"q{h}")

            # ---------------- phase 3: attention ------------------------
            yT = bigpool.tile([P, NH, QR], bf16, tag="yT")

            def load_wo_slab(s):
                # shares the xc pool; slab s=1 evicts xo_sb (read-dep safe)
                w3 = xcpool.tile([P, NCT, QR], bf16, tag="xc", name=f"wo{s}")
                for gr in range(4):
                    nc.sync.dma_start(
                        out=w3[:, 4 * gr:4 * gr + 4, :],
                        in_=wo[s][:, 4 * gr:4 * gr + 4, :],
                    )
                return w3

            w3s = {0: load_wo_slab(0), 1: load_wo_slab(1)}

            tail = []

            def emit_tail():
                yt, den, h = tail.pop(0)
                rinv = smallp.tile([P, QR], f32, tag="rq", name=f"rq{h}")
                nc.vector.reciprocal_approx_fast(rinv, den)
                nc.vector.tensor_tensor(yT[:, h, :], yt, rinv, op=OP.mult)

            def emit_ent(ent, yt, den, kv):
                """single: per-kt den + PV; pair: pair-summed den + 2 PV."""
                kind, m, n, pa, pb = ent
                if kind == "s":
                    kt = m
                    last = False
                    nc.tensor.matmul(
                        den[:, 0:n], ones128, pa,
                        start=(kt == 0), stop=False, skip_group_check=True,
                    )
                    nc.tensor.matmul(
                        yt[:, 0:n], vA[:, kv, kt, :], pa,
                        start=(kt == 0), stop=False, skip_group_check=True,
                    )
                else:
                    last = m == 7
                    pad = paddp.tile([P, QR], bf16, tag="pa", name=f"pad{m}")
                    nc.vector.tensor_tensor(pad[:, 0:n], pa, pb, op=OP.add)
                    nc.tensor.matmul(
                        den[:, 0:n], ones128, pad[:, 0:n],
                        start=False, stop=last, skip_group_check=True,
                    )
                    nc.tensor.matmul(
                        yt[:, 0:n], vA[:, kv, 2 * m, :], pa,
                        start=False, stop=False, skip_group_check=True,
                    )
                    nc.tensor.matmul(
                        yt[:, 0:n], vA[:, kv, 2 * m + 1, :], pb,
                        start=False, stop=last, skip_group_check=True,
                    )

            for h in range(NH):
                kv = h // (NH // NKV)
                yt = psY.tile([P, QR], f32, tag="Y", name=f"yt{h}")
                den = psD.tile([P, QR], f32, tag="D", name=f"den{h}")
                dq = []
                # kt 0..7: one kt per psum tile, causal mask accumulated
                # on the PE, pipeline depth 4
                for kt in range(8):
                    n = 64 * (8 - kt // 2)
                    i = (15 - kt) // 2
                    S = psS.tile([P, QR], f32, tag="S", name=f"S{h}_{kt}")
                    nc.tensor.matmul(
                        S[:, 0:n],
                        kT[:, kv, kt * P:(kt + 1) * P],
                        qT[:, h, 0:n],
                        start=True, stop=False, skip_group_check=True,
                    )
                    nc.tensor.matmul(
                        S[:, n - 64:n], ident, mk[:, i, kt % 2, :],
                        start=False, stop=True, skip_group_check=True,
                    )
                    if kt == 0 and tail:
                        emit_tail()
                    pt = ptpool.tile([P, QR], bf16, tag="pt", name=f"pt{h}_{kt}")
                    nc.scalar.activation(pt[:, 0:n], S[:, 0:n], AF.Exp)
                    dq.append(("s", kt, n, pt[:, 0:n], None))
                    if len(dq) > 3:
                        emit_ent(dq.pop(0), yt, den, kv)
                # kt 8..15: both kt of a pair packed into one psum bank
                for m in range(4, 8):
                    n = 64 * (8 - m)
                    i = 7 - m
                    S = psS.tile([P, QR], f32, tag="S", name=f"S{h}_{m}p")
                    for t in range(2):
                        nc.tensor.matmul(
                            S[:, t * n:(t + 1) * n],
                            kT[:, kv, (2 * m + t) * P:(2 * m + t + 1) * P],
                            qT[:, h, 0:n],
                            start=True, stop=False, skip_group_check=True,
                        )
                        nc.tensor.matmul(
                            S[:, (t + 1) * n - 64:(t + 1) * n], ident,
                            mk[:, i, t, :],
                            start=False, stop=True, skip_group_check=True,
                        )
                    pt = ptpool.tile([P, QR], bf16, tag="pt", name=f"pt{h}_{m}p")
                    nc.scalar.activation(pt[:, 0:2 * n], S[:, 0:2 * n], AF.Exp)
                    dq.append(("p", m, n, pt[:, 0:n], pt[:, n:2 * n]))
                    if len(dq) > 3:
                        emit_ent(dq.pop(0), yt, den, kv)
                while dq:
                    emit_ent(dq.pop(0), yt, den, kv)
                tail.append((yt, den, h))
            emit_tail()

            # ---------------- phase 4: output projection ----------------
            for s in range(4):
                w3 = w3s.pop(s)
                if s + 2 < 4:
                    w3s[s + 2] = load_wo_slab(s + 2)
                for rt in range(4):
                    ps = psS.tile([P, QR], f32, tag="S", name=f"o{s}_{rt}")
                    for h in range(NH):
                        nc.tensor.matmul(
                            ps, yT[:, h, rt * P:(rt + 1) * P], w3[:, h, :],
                            start=(h == 0), stop=(h == NH - 1),
                        )
                    ot = outpool.tile([P, QR], f32, tag="ot", name=f"ot{s}_{rt}")
                    nc.vector.tensor_copy(ot, ps)
                    nc.gpsimd.dma_start(
                        out=yo[rt * P:(rt + 1) * P, s * QR:(s + 1) * QR], in_=ot
                    )

    nc.compile()
    return nc


def _get_nc():
    if "nc" not in _CACHE:
        _CACHE["nc"] = _build()
    return _CACHE["nc"]


def _prep_in_maps(x, cos, sin, wq, wk, wv, wo):
    x = np.asarray(x, np.float32)
    cosr = np.asarray(cos, np.float32).reshape(T, HD // 2)
    sinr = np.asarray(sin, np.float32).reshape(T, HD // 2)
    # weight layouts match the SBUF tiles exactly -> contiguous DMAs
    wqb = np.ascontiguousarray(
        np.asarray(wq, np.float32).reshape(NCT, P, NH, HD)
        .transpose(2, 1, 0, 3).astype(BF16))
    wob = np.ascontiguousarray(
        np.asarray(wo, np.float32).reshape(NCT, P, 4, QR)
        .transpose(2, 1, 0, 3).astype(BF16))
    wkf = np.asarray(wk, np.float32)
    wvf = np.asarray(wv, np.float32)

    csf = np.ascontiguousarray(np.concatenate([cosr.T, sinr.T], axis=0))
    snf = np.ascontiguousarray(np.concatenate([sinr.T, cosr.T], axis=0))
    rpA_np, rpB_np = _rope_mats()

    maps = []
    for core in range(8):
        b, g = core // 4, core % 4
        qr = _qrows(g)
        xT_b = x[b].T
        maps.append({
            "xoT": np.ascontiguousarray(
                xT_b[:, qr].reshape(NCT, P, QR).transpose(1, 0, 2).astype(BF16)),
            "xfT": np.ascontiguousarray(
                xT_b.reshape(NCT, P, 4, QR).transpose(2, 1, 0, 3).astype(BF16)),
            "cso": np.ascontiguousarray(csf[:, qr]),
            "sno": np.ascontiguousarray(snf[:, qr]),
            "csf": csf,
            "snf": snf,
            "wq": wqb,
            "wks": np.ascontiguousarray(
                wkf[:, g * HD:(g + 1) * HD].reshape(NCT, P, HD)
                .transpose(1, 0, 2).astype(BF16)),
            "wvs": np.ascontiguousarray(
                wvf[:, g * HD:(g + 1) * HD].reshape(NCT, P, HD)
                .transpose(1, 0, 2).astype(BF16)),
            "wo": wob,
            "msk": np.ascontiguousarray(_mask01(g).transpose(2, 0, 1, 3)),
            "rpA": rpA_np,
            "rpB": rpB_np,
        })
    return maps


def kernel(x, cos, sin, wq, wk, wv, wo):
    from concourse.bass_utils import run_bass_kernel_spmd

    nc = _get_nc()
    maps = _prep_in_maps(x, cos, sin, wq, wk, wv, wo)
    _CACHE["in_maps"] = maps
    res = run_bass_kernel_spmd(nc, maps, list(range(8)))
    y = np.empty((B, T, C), np.float32)
    for core in range(8):
        b, g = core // 4, core % 4
        y[b][_qrows(g)] = res.results[core]["yo"]
    return y
